# revision 44
# baseline (speedup 1.0000x reference)
"""Multi-head self-attention (B=16, N=1024, D=768, H=12) on 8 TRN2 NeuronCores.

Data-parallel over batch (2 batches per core, weights replicated, no
collectives). Per core, one fused Bass/Tile kernel:

  x --8 chunked interleaved DMAs--> x6 [128, 8*768] (token 8p+t on
      partition p, slot t; attention is permutation-invariant over tokens,
      so the interleave is only undone at the output DMA)
  x6 --f16 cast + PE transpose--> xT [d, tok]
  QT/KT = (W_qkv^T x^T + b) in [col, tok] layout (f16)
  V_aug = [x W_v | ones-col per head]  [tok, 12*65] (f16)
  per head: S^T[m,n] = K Q^T (PE), E = exp(S^T*scale) (ACT, [128,1024]),
      O^T = V_aug^T E (PE; row 64 = softmax denominator via the ones
      column -- no max subtraction needed, scores are O(1)).
      normalize: recip_approx_fast straight from PSUM row 64,
      DMA-broadcast the reciprocal row (f32), one DVE mul reading PSUM.
  out = attnT^T W_proj, bias (W_proj^T b_v + b_proj) added by the DVE
      during the PSUM->SBUF copy from a DMA-broadcast bias tile
      (V-bias folded through softmax since rows of A sum to 1)

All matmul operands f16 (1 cycle/row; fp32/f32r run 2-pass fp32_mode=HIGH
at 1/4 rate and break HAM warm-up -- measured). PSUM accumulation is f32.

Scheduling: engines execute fixed in-order streams, so emission order IS
the schedule. Startup pipelines chunked x DMAs -> transposes -> V/QKV so
batch-0 attention starts ~30us earlier than a phase-serial ordering.
Attention loops are nh-outer so each batch's first-half projection becomes
filler work inside its own attention window; b1's QKV pairs 1-5 are
deferred into b1's window to keep the PE fed there (stalled PE breaks the
LDWEIGHTS pull-ahead and inflates score MMs from 213ns to 318ns).
"""

import numpy as np

_CACHE: dict = {}

P = 128
BL, N, D, H, HD = 2, 1024, 768, 12, 64
D3 = 3 * D
SCALE = float(HD) ** -0.5


def _build():
    import concourse.mybir as mybir
    import concourse.tile as tile
    from concourse import bacc
    from concourse.masks import make_identity

    dt = mybir.dt
    F32, F16 = dt.float32, dt.float16
    AF = mybir.ActivationFunctionType

    nc = bacc.Bacc("TRN2", target_bir_lowering=False, debug=False)
    x_d = nc.dram_tensor("x", [BL, N, D], F32, kind="ExternalInput").ap()
    wqkv_d = nc.dram_tensor("W_qkv", [D, D3], F32, kind="ExternalInput").ap()
    bqkv_d = nc.dram_tensor("b_qkv", [D3], F32, kind="ExternalInput").ap()
    wproj_d = nc.dram_tensor("W_proj", [D, D], F32, kind="ExternalInput").ap()
    bproj_d = nc.dram_tensor("b_proj", [D], F32, kind="ExternalInput").ap()
    out_d = nc.dram_tensor("out", [BL, N, D], F32, kind="ExternalOutput").ap()
    # token-interleaved views: partition p, slot t <-> token 8p+t
    x_il = x_d.rearrange("b (p i) d -> b p (i d)", p=P)       # [2, 128, 6144]
    out_il = out_d.rearrange("b (p i) d -> b i p d", p=P)     # [2, 8, 128, 768]

    with tile.TileContext(nc) as tc:
        with tc.tile_pool(name="sb", bufs=1) as sb, \
             tc.tile_pool(name="dp", bufs=1, space="DRAM") as dp, \
             tc.tile_pool(name="ps", bufs=2, space="PSUM") as ps:

            # ---------- constants ----------
            ident = sb.tile([P, P], F16, tag="ident", bufs=1, name="ident")
            make_identity(nc, ident[:])
            ones_h = sb.tile([P, P], F16, tag="ones_h", bufs=1, name="ones_h")
            nc.vector.memset(ones_h[:], 1.0)

            # ---------- DMA issue order, by first use: V columns of W_qkv,
            # x chunks 0-3, the j=0/j=6 W columns (QKV pair 0), x chunks
            # 4-7, then the remaining W columns (pairs 1-5, used as b0-
            # window fillers so they may land late).
            x6 = {}
            x6[0] = sb.tile([P, 8 * D], F32, tag="x6", bufs=1, name="x6")
            for c in range(2):
                nc.sync.dma_start(x6[0][:, D * c:D * (c + 1)],
                                  x_il[0][:, D * c:D * (c + 1)])

            wv_stgs = []
            for d in range(6):
                stg = sb.tile([P, D], F32, tag="wstage", bufs=2,
                              name="wvstg")
                nc.sync.dma_start(stg[:],
                                  wqkv_d[P * d:P * (d + 1), 2 * D:3 * D])
                wv_stgs.append(stg)

            for c in range(2, 6):
                nc.sync.dma_start(x6[0][:, D * c:D * (c + 1)],
                                  x_il[0][:, D * c:D * (c + 1)])

            wj_stgs = []
            for d in range(6):
                for j in (0, 6):
                    stg = sb.tile([P, P], F32, tag="wjstage", bufs=3,
                                  name="wjstg")
                    nc.sync.dma_start(
                        stg[:], wqkv_d[P * d:P * (d + 1),
                                       P * j:P * (j + 1)])
                    wj_stgs.append(stg)

            for c in range(6, 8):
                nc.sync.dma_start(x6[0][:, D * c:D * (c + 1)],
                                  x_il[0][:, D * c:D * (c + 1)])

            # bias + bproj staging on the scalar DMA queue
            bstg = sb.tile([18, P], F32, tag="bstg", bufs=1, name="bstg")
            nc.scalar.dma_start(bstg[:], bqkv_d.rearrange("(j p) -> j p", p=P))
            bproj_row = sb.tile([1, D], F32, tag="bproj_row", bufs=1,
                                name="bproj_row")
            nc.scalar.dma_start(bproj_row[:], bproj_d.unsqueeze(0))

            # ---------- PE warm-up: ~4us of dummy matmuls flips HAM to 8/8
            # (transposes run in transpose-mode, which does not warm HAM)
            warm_h = sb.tile([P, 512], F16, tag="e", bufs=3, name="warm_h")
            nc.vector.memset(warm_h[:], 0.0)
            for wi in range(10):
                wps = ps.tile([P, 512], F32, tag="mm", bufs=2, name="wps")
                nc.tensor.matmul(wps[:], ones_h[:, 0:P], warm_h[:],
                                 start=True, stop=True)

            # ---------- W_qkv casts: V cols + j0/j6 on the (startup-idle)
            # scalar engine; the rest on DVE later (emission-order note:
            # these reads must precede the wrest DMAs' tag reuse).
            wq_h = []
            for d in range(6):
                t = sb.tile([P, D3], F16, tag=f"wqkv{d}", bufs=1,
                            name=f"wqkv{d}")
                nc.scalar.copy(t[:, 2 * D:3 * D], wv_stgs[d][:])
                for ji, j in enumerate((0, 6)):
                    nc.scalar.copy(t[:, P * j:P * (j + 1)],
                                   wj_stgs[2 * d + ji][:])
                wq_h.append(t)

            # remaining W_qkv columns (lands late; cast on DVE at startup
            # end, consumed by pair-1..5 QKV groups inside b0's window)
            wr_stgs = []
            for d in range(6):
                for rg in range(2):
                    c0 = (P, 7 * P)[rg]
                    stg = sb.tile([P, 5 * P], F32, tag="wstage", bufs=2,
                                  name="wrstg")
                    nc.sync.dma_start(
                        stg[:], wqkv_d[P * d:P * (d + 1), c0:c0 + 5 * P])
                    wr_stgs.append(stg)

            # ---------- bias pipeline (DVE + one PE transpose) ----------
            bstg_h = sb.tile([18, P], F16, tag="bstg_h", bufs=1, name="bstg_h")
            nc.vector.tensor_copy(bstg_h[:], bstg[:])
            btp = ps.tile([P, 18], F16, tag="mm", bufs=2, name="btp")
            nc.tensor.transpose(btp[:], bstg_h[:], ident[0:18, 0:18])
            bqkvT = sb.tile([P, 18], F32, tag="bqkvT", bufs=1, name="bqkvT")
            nc.vector.tensor_copy(bqkvT[:], btp[:])
            bv_h = sb.tile([P, 6], F16, tag="bv_h", bufs=1, name="bv_h")
            nc.vector.tensor_copy(bv_h[:], btp[:, 12:18])

            xT = {b: [sb.tile([P, N], F16, tag=f"xT{b}_{j}", bufs=1,
                              name=f"xT{b}_{j}") for j in range(6)]
                  for b in range(BL)}

            def emit_transpose_chunk(b, t):
                xh = sb.tile([P, D], F16, tag="xh", bufs=1, name="xh")
                nc.vector.tensor_copy(xh[:], x6[b][:, D * t:D * (t + 1)])
                for j in range(6):
                    tp = ps.tile([P, P], F16, tag="mm", bufs=2, name="tp")
                    nc.tensor.transpose(tp[:], xh[:, P * j:P * (j + 1)],
                                        ident[:])
                    nc.vector.tensor_copy(xT[b][j][:, P * t:P * (t + 1)],
                                          tp[:])

            # ---------- result tiles ----------
            qk = {b: [sb.tile([P, N], F16, tag=f"qk{b % 2}_{j}", bufs=1,
                              name=f"qk{j}") for j in range(12)]
                  for b in range(BL)}
            v = {b: [sb.tile([P, 12 * 65], F16, tag=f"v{b % 2}_{t}", bufs=1,
                             name=f"v{t}") for t in range(8)]
                 for b in range(BL)}
            at = {b: [sb.tile([P, N], F16, tag=f"at{j}", bufs=1,
                              name=f"at{j}") for j in range(6)]
                  for b in range(BL)}

            def emit_qkv_group(b, j, nh):
                qps = ps.tile([P, 512], F32, tag="mm", bufs=2, name="qps")
                for d in range(6):
                    nc.tensor.matmul(qps[:], wq_h[d][:, P * j:P * (j + 1)],
                                     xT[b][d][:, 512 * nh:512 * (nh + 1)],
                                     start=(d == 0), stop=(d == 5))
                nc.vector.tensor_scalar_add(
                    qk[b][j][:, 512 * nh:512 * (nh + 1)], qps[:],
                    bqkvT[:, j:j + 1])

            def emit_v_group(b, t, ci):
                c0, cw = ((0, 512), (512, 256))[ci]
                v3 = v[b][t].rearrange("p (h c) -> p h c", c=65)
                if ci == 0:
                    nc.vector.tensor_copy(v3[:, :, 64:65],
                                          ones_h[:, 0:12].unsqueeze(2))
                vps = ps.tile([P, 512], F32, tag="mm", bufs=2, name="vps")
                for d in range(6):
                    nc.tensor.matmul(vps[:, 0:cw],
                                     xT[b][d][:, P * t:P * (t + 1)],
                                     wq_h[d][:, 2 * D + c0:2 * D + c0 + cw],
                                     start=(d == 0), stop=(d == 5))
                nc.vector.tensor_copy(
                    v3[:, (c0 // HD):((c0 + cw) // HD), 0:HD],
                    vps[:, 0:cw].rearrange("p (h c) -> p h c", c=HD))

            # wp_h / bfinal state -- filled by fillers inside b0's window
            wp_h = []
            wpstgs = []
            bfb = sb.tile([P, D], F16, tag="bfb", bufs=1, name="bfb")

            def emit_wproj_dmas():
                for d in range(6):
                    stg = sb.tile([P, D], F32, tag="wstage", bufs=2,
                                  name="wpstg")
                    nc.scalar.dma_start(stg[:],
                                        wproj_d[P * d:P * (d + 1), :])
                    wpstgs.append(stg)

            def emit_wproj_cast(d):
                t = sb.tile([P, D], F16, tag=f"wproj{d}", bufs=1,
                            name=f"wproj{d}")
                nc.vector.tensor_copy(t[:], wpstgs[d][:])
                wp_h.append(t)

            def emit_bfinal():
                # b_final = W_proj^T b_v + b_proj  [1, 768] -> broadcast
                # to [128, 768] f16 via a DRAM bounce so the proj-output
                # copy can add it on the DVE (replaces 32 K=1 PE matmuls).
                bfinal_f = sb.tile([1, D], F32, tag="bfinal", bufs=1,
                                   name="bfinal")
                for c0, cw in ((0, 512), (512, 256)):
                    bf_ps = ps.tile([1, 512], F32, tag="mm", bufs=2,
                                    name="bf_ps")
                    for d in range(6):
                        nc.tensor.matmul(bf_ps[:, 0:cw], bv_h[:, d:d + 1],
                                         wp_h[d][:, c0:c0 + cw],
                                         start=(d == 0), stop=(d == 5))
                    nc.vector.tensor_add(bfinal_f[:, c0:c0 + cw],
                                         bf_ps[0:1, 0:cw],
                                         bproj_row[:, c0:c0 + cw])
                bfinal_h = sb.tile([1, D], F16, tag="bfinal_h", bufs=1,
                                   name="bfinal_h")
                nc.vector.tensor_copy(bfinal_h[:], bfinal_f[:])
                bf_d = dp.tile([1, D], F16, tag="bf_d", bufs=1, name="bf_d")
                nc.sync.dma_start(bf_d[:], bfinal_h[:])
                nc.sync.dma_start(bfb[:], bf_d[:].to_broadcast((P, D)))

            def emit_proj_half(b, t, ci):
                c0, cw = ((0, 512), (512, 256))[ci]
                pps = ps.tile([P, 512], F32, tag="mm", bufs=2, name="pps")
                for d in range(6):
                    nc.tensor.matmul(pps[:, 0:cw],
                                     at[b][d][:, P * t:P * (t + 1)],
                                     wp_h[d][:, c0:c0 + cw],
                                     start=(d == 0), stop=(d == 5))
                osb = sb.tile([P, 512], F32, tag="outs", bufs=2, name="osb")
                nc.vector.tensor_add(osb[:, 0:cw], pps[:, 0:cw],
                                     bfb[:, c0:c0 + cw])
                nc.sync.dma_start(out_il[b, t][:, c0:c0 + cw], osb[:, 0:cw])

            def pop_fillers(fillers, k=2):
                n = 0
                while fillers and n < k:
                    fillers.pop(0)()
                    n += 1

            # one-unit-deep queue of deferred normalize multiplies: the
            # reciprocal-broadcast DMA round trip gets a full unit of
            # latency slack instead of blocking the DVE stream in-line.
            pend_muls = []

            def flush_muls():
                while pend_muls:
                    pend_muls.pop(0)()

            def emit_unit(b, jp, nh, fillers, popk=2, tail=False):
                qt, kt = qk[b][jp], qk[b][6 + jp]
                n0 = 512 * nh
                ot = [ps.tile([65, 512], F32, tag="ot", bufs=2,
                              name="otps") for _ in range(2)]
                def emit_avs(pend):
                    for pm, pe_ in pend:
                        for hh in range(2):
                            h = 2 * jp + hh
                            nc.tensor.matmul(
                                ot[hh][:],
                                v[b][pm][:, 65 * h:65 * h + 65],
                                pe_[:, 512 * hh:512 * (hh + 1)],
                                start=(pm == 0), stop=(pm == 7))

                # m-blocks processed in pairs: the two score matmul pairs
                # are emitted back-to-back so the second pair's row-tiled
                # LDWEIGHTS pulls ahead behind a score MM (disjoint row
                # groups) instead of serializing behind a full-row AV MM.
                # AVs for the previous pair go before this pair's exps so
                # the e-buffer (bufs=3) WAR stays in emission order.
                pend = []
                for mp in range(4):
                    spss = []
                    for mi in range(2):
                        m = 2 * mp + mi
                        sps = ps.tile([P, N], F32, tag="s", bufs=2,
                                      name="sps")
                        for hh in range(2):
                            r0, r1 = HD * hh, HD * (hh + 1)
                            nc.tensor.matmul(
                                sps[:, 512 * hh:512 * (hh + 1)],
                                kt[r0:r1, P * m:P * (m + 1)],
                                qt[r0:r1, n0:n0 + 512],
                                start=True, stop=True)
                        spss.append((m, sps))
                    emit_avs(pend)
                    pend = []
                    for m, sps in spss:
                        e = sb.tile([P, N], F16, tag="e", bufs=3, name="e")
                        nc.scalar.activation(e[:], sps[:], AF.Exp,
                                             scale=SCALE)
                        pend.append((m, e))
                    if mp in (1, 2):
                        pop_fillers(fillers, popk)
                emit_avs(pend)
                # end-of-unit fillers come BEFORE the serial normalize
                # chain so their PSUM-group-closing DVE ops aren't queued
                # behind it (the next unit's filler matmuls WAR on them).
                pop_fillers(fillers, popk)
                # flush the PREVIOUS unit's deferred muls first: their bc
                # broadcasts landed a unit ago, and the bc/u_sb buffers
                # (bufs=2) are WAR-reused by this unit right after.
                flush_muls()

                def recip_chain(hh):
                    dr_f = sb.tile([1, 512], F32, tag="dr_f", bufs=1,
                                   name="dr_f")
                    nc.vector.tensor_copy(dr_f[:], ot[hh][64:65, :])
                    rr_f = sb.tile([1, 512], F32, tag="rr_f", bufs=1,
                                   name="rr_f")
                    nc.vector.reciprocal_approx_fast(out=rr_f[:],
                                                     in_=dr_f[:])
                    rr_h = sb.tile([1, 512], F16, tag="rr_h", bufs=1,
                                   name="rr_h")
                    nc.vector.tensor_copy(rr_h[:], rr_f[:])
                    rr_d = dp.tile([1, 512], F16, tag="rr_d", bufs=4,
                                   name="rr_d")
                    nc.sync.dma_start(rr_d[:], rr_h[:])
                    bc_h = sb.tile([HD, 512], F16, tag="bc_h", bufs=2,
                                   name="bc_h")
                    nc.sync.dma_start(bc_h[:],
                                      rr_d[:].to_broadcast((HD, 512)))
                    return bc_h

                def u_copy(hh):
                    # copy O^T out of PSUM promptly -- ot has no cross-
                    # unit slack (2 allocs/unit, bufs=2), so a deferred
                    # PSUM read would stall the next unit's first AV.
                    u_sb = sb.tile([HD, 512], F16, tag="u_sb", bufs=2,
                                   name="u_sb")
                    nc.vector.tensor_copy(u_sb[:], ot[hh][0:HD, :])
                    return u_sb

                if tail:
                    # window-final unit: the tail work waits on these
                    # muls, so skip the DMA broadcast round trip entirely
                    # -- broadcast the reciprocal row with a K=1 matmul
                    # on the (idle-at-tail) PE and multiply immediately.
                    for hh in range(2):
                        r0, r1 = HD * hh, HD * (hh + 1)
                        dr_f = sb.tile([1, 512], F32, tag="dr_f", bufs=1,
                                       name="dr_f")
                        nc.vector.tensor_copy(dr_f[:], ot[hh][64:65, :])
                        rr_f = sb.tile([1, 512], F32, tag="rr_f", bufs=1,
                                       name="rr_f")
                        nc.vector.reciprocal_approx_fast(out=rr_f[:],
                                                         in_=dr_f[:])
                        rr_h = sb.tile([1, 512], F16, tag="rr_h", bufs=1,
                                       name="rr_h")
                        nc.vector.tensor_copy(rr_h[:], rr_f[:])
                        bc_ps = ps.tile([HD, 512], F32, tag="mm", bufs=2,
                                        name="bc_ps")
                        nc.tensor.matmul(bc_ps[:], ones_h[0:1, 0:HD],
                                         rr_h[:], start=True, stop=True)
                        u_sb = u_copy(hh)
                        nc.vector.tensor_mul(
                            at[b][jp][r0:r1, n0:n0 + 512],
                            u_sb[:], bc_ps[:])
                else:
                    us, bcs = [], []
                    for hh in range(2):
                        us.append(u_copy(hh))
                        bcs.append(recip_chain(hh))
                    for hh in range(2):
                        r0, r1 = HD * hh, HD * (hh + 1)
                        pend_muls.append(
                            lambda u=us[hh], bc=bcs[hh], r0=r0, r1=r1:
                            nc.vector.tensor_mul(
                                at[b][jp][r0:r1, n0:n0 + 512],
                                u[:], bc[:]))

            # ---------- startup: b0 transposes + QKV pair 0 + V(0,1);
            # V(2..7) ride as unit-0 fillers (popk=4 keeps each V(t)
            # ahead of the AV matmul that consumes it)
            for t in range(8):
                emit_transpose_chunk(0, t)
            for t in range(8):
                emit_v_group(0, t, 0)
                emit_v_group(0, t, 1)
            emit_qkv_group(0, 0, 0)
            emit_qkv_group(0, 6, 0)
            emit_qkv_group(0, 6, 1)

            # cast the late W_qkv columns on DVE (reads must be emitted
            # before the wproj DMAs reuse the wstage buffers)
            for d in range(6):
                for rg in range(2):
                    c0 = (P, 7 * P)[rg]
                    nc.vector.tensor_copy(wq_h[d][:, c0:c0 + 5 * P],
                                          wr_stgs[2 * d + rg][:])

            # issue remaining bulk DMAs now (land well before their use)
            emit_wproj_dmas()
            x6[1] = sb.tile([P, 8 * D], F32, tag="x6", bufs=1, name="x6")
            for c in range(8):
                nc.sync.dma_start(x6[1][:, D * c:D * (c + 1)],
                                  x_il[1][:, D * c:D * (c + 1)])

            # ---------- b0 attention, nh-outer ----------
            # nh0 fillers: pair jp+1's q/k groups front-loaded so each
            # pair's inputs are emitted a unit ahead of use (per-engine
            # streams are in-order: a consumer emitted before its producer
            # would deadlock), then q(*,nh1) groups, wproj casts, bfinal.
            q_nh0 = []
            for jpn in range(1, 6):
                q_nh0 += [
                    lambda j=jpn: emit_qkv_group(0, j, 0),
                    lambda j=6 + jpn: emit_qkv_group(0, j, 0),
                    lambda j=6 + jpn: emit_qkv_group(0, j, 1),
                    lambda j=jpn - 1: emit_qkv_group(0, j, 1),
                ]
            q_nh0 += [lambda: emit_qkv_group(0, 5, 1)]
            q_nh0 += [lambda d=d: emit_wproj_cast(d) for d in range(6)]
            q_nh0 += [emit_bfinal]
            for jp in range(6):
                emit_unit(0, jp, 0, q_nh0, tail=(jp == 5))
            flush_muls()
            while q_nh0:
                q_nh0.pop(0)()

            # nh1 fillers: b1 transposes + V(1) interleaved per chunk,
            # b1 QKV pair 0, then b0 proj for nh0 token chunks (0-3).
            q_nh1 = []
            for t in range(8):
                q_nh1 += [lambda t=t: emit_transpose_chunk(1, t)]
                q_nh1 += [lambda t=t, ci=ci: emit_v_group(1, t, ci)
                          for ci in range(2)]
            q_nh1 += [lambda: emit_qkv_group(1, 0, 0),
                      lambda: emit_qkv_group(1, 6, 0),
                      lambda: emit_qkv_group(1, 6, 1)]
            q_nh1 += [lambda t=t, ci=ci: emit_proj_half(0, t, ci)
                      for t in range(4) for ci in range(2)]
            for jp in range(6):
                emit_unit(0, jp, 1, q_nh1, tail=(jp == 5))
            flush_muls()
            while q_nh1:
                q_nh1.pop(0)()

            # ---------- b1 attention, nh-outer ----------
            # nh0 fillers: b1 QKV pairs 1-5 staged one unit ahead, then b0
            # proj for nh1 token chunks (popped at units 2+, after which
            # the at[0] reads are done so b1's at writes don't stall).
            q1_nh0 = [lambda: emit_qkv_group(1, 0, 1)]
            for jpn in range(1, 6):
                q1_nh0 += [
                    lambda j=jpn: emit_qkv_group(1, j, 0),
                    lambda j=6 + jpn: emit_qkv_group(1, j, 0),
                    lambda j=6 + jpn: emit_qkv_group(1, j, 1),
                    lambda j=jpn: emit_qkv_group(1, j, 1),
                ]
            q1_nh0 += [lambda t=t, ci=ci: emit_proj_half(0, t, ci)
                       for t in range(4, 8) for ci in range(2)]
            for jp in range(6):
                emit_unit(1, jp, 0, q1_nh0, tail=(jp == 5))
            flush_muls()
            while q1_nh0:
                q1_nh0.pop(0)()

            # nh1 fillers: b1 proj for nh0 token chunks (0-3).
            q1_nh1 = [lambda t=t, ci=ci: emit_proj_half(1, t, ci)
                      for t in range(4) for ci in range(2)]
            for jp in range(6):
                emit_unit(1, jp, 1, q1_nh1, tail=(jp == 5))
            flush_muls()
            while q1_nh1:
                q1_nh1.pop(0)()

            # ---------- tail: b1 proj for nh1 token chunks ----------
            for t in range(4, 8):
                for ci in range(2):
                    emit_proj_half(1, t, ci)
    nc.compile()
    return nc


def _get_nc():
    if "nc" not in _CACHE:
        _CACHE["nc"] = _build()
    return _CACHE["nc"]


def kernel(x, W_qkv, b_qkv, W_proj, b_proj):
    from concourse.bass_utils import run_bass_kernel_spmd

    nc = _get_nc()
    x = np.ascontiguousarray(x, dtype=np.float32)
    in_maps = [
        {
            "x": x[2 * i:2 * i + 2],
            "W_qkv": np.asarray(W_qkv, dtype=np.float32),
            "b_qkv": np.asarray(b_qkv, dtype=np.float32),
            "W_proj": np.asarray(W_proj, dtype=np.float32),
            "b_proj": np.asarray(b_proj, dtype=np.float32),
        }
        for i in range(8)
    ]
    res = run_bass_kernel_spmd(nc, in_maps, core_ids=list(range(8)))
    return np.concatenate([r["out"] for r in res.results], axis=0)


# revision 46
# speedup vs baseline: 1.0195x; 1.0195x over previous
"""Multi-head self-attention (B=16, N=1024, D=768, H=12) on 8 TRN2 NeuronCores.

Data-parallel over batch (2 batches per core, weights replicated, no
collectives). Per core, one fused Bass/Tile kernel:

  x --8 chunked interleaved DMAs--> x6 [128, 8*768] (token 8p+t on
      partition p, slot t; attention is permutation-invariant over tokens,
      so the interleave is only undone at the output DMA)
  x6 --f16 cast + PE transpose--> xT [d, tok]
  QT/KT = (W_qkv^T x^T + b) in [col, tok] layout (f16)
  V_aug = [x W_v | ones-col per head]  [tok, 12*65] (f16)
  per head: S^T[m,n] = K Q^T (PE), E = exp(S^T*scale) (ACT, [128,1024]),
      O^T = V_aug^T E (PE; row 64 = softmax denominator via the ones
      column -- no max subtraction needed, scores are O(1)).
      normalize: recip_approx_fast straight from PSUM row 64,
      DMA-broadcast the reciprocal row (f32), one DVE mul reading PSUM.
  out = attnT^T W_proj, bias (W_proj^T b_v + b_proj) added by the DVE
      during the PSUM->SBUF copy from a DMA-broadcast bias tile
      (V-bias folded through softmax since rows of A sum to 1)

All matmul operands f16 (1 cycle/row; fp32/f32r run 2-pass fp32_mode=HIGH
at 1/4 rate and break HAM warm-up -- measured). PSUM accumulation is f32.

Scheduling: engines execute fixed in-order streams, so emission order IS
the schedule. Startup pipelines chunked x DMAs -> transposes -> V/QKV so
batch-0 attention starts ~30us earlier than a phase-serial ordering.
Attention loops are nh-outer so each batch's first-half projection becomes
filler work inside its own attention window; b1's QKV pairs 1-5 are
deferred into b1's window to keep the PE fed there (stalled PE breaks the
LDWEIGHTS pull-ahead and inflates score MMs from 213ns to 318ns).
"""

import numpy as np

_CACHE: dict = {}

P = 128
BL, N, D, H, HD = 2, 1024, 768, 12, 64
D3 = 3 * D
SCALE = float(HD) ** -0.5


def _build():
    import concourse.mybir as mybir
    import concourse.tile as tile
    from concourse import bacc
    from concourse.masks import make_identity

    dt = mybir.dt
    F32, F16 = dt.float32, dt.float16
    AF = mybir.ActivationFunctionType

    nc = bacc.Bacc("TRN2", target_bir_lowering=False, debug=False)
    x_d = nc.dram_tensor("x", [BL, N, D], F32, kind="ExternalInput").ap()
    wqkv_d = nc.dram_tensor("W_qkv", [D, D3], F32, kind="ExternalInput").ap()
    bqkv_d = nc.dram_tensor("b_qkv", [D3], F32, kind="ExternalInput").ap()
    wproj_d = nc.dram_tensor("W_proj", [D, D], F32, kind="ExternalInput").ap()
    bproj_d = nc.dram_tensor("b_proj", [D], F32, kind="ExternalInput").ap()
    out_d = nc.dram_tensor("out", [BL, N, D], F32, kind="ExternalOutput").ap()
    # token-interleaved views: partition p, slot t <-> token 8p+t
    x_il = x_d.rearrange("b (p i) d -> b p (i d)", p=P)       # [2, 128, 6144]
    out_il = out_d.rearrange("b (p i) d -> b i p d", p=P)     # [2, 8, 128, 768]

    with tile.TileContext(nc) as tc:
        with tc.tile_pool(name="sb", bufs=1) as sb, \
             tc.tile_pool(name="dp", bufs=1, space="DRAM") as dp, \
             tc.tile_pool(name="ps", bufs=2, space="PSUM") as ps:

            # ---------- constants ----------
            ident = sb.tile([P, P], F16, tag="ident", bufs=1, name="ident")
            make_identity(nc, ident[:])
            ones_h = sb.tile([P, P], F16, tag="ones_h", bufs=1, name="ones_h")
            nc.vector.memset(ones_h[:], 1.0)

            # ---------- DMA issue order, by first use: V columns of W_qkv,
            # x chunks 0-3, the j=0/j=6 W columns (QKV pair 0), x chunks
            # 4-7, then the remaining W columns (pairs 1-5, used as b0-
            # window fillers so they may land late).
            x6 = {}
            x6[0] = sb.tile([P, 8 * D], F32, tag="x6", bufs=1, name="x6")
            for c in range(2):
                nc.sync.dma_start(x6[0][:, D * c:D * (c + 1)],
                                  x_il[0][:, D * c:D * (c + 1)])

            wv_stgs = []
            for d in range(6):
                stg = sb.tile([P, D], F32, tag="wstage", bufs=2,
                              name="wvstg")
                nc.sync.dma_start(stg[:],
                                  wqkv_d[P * d:P * (d + 1), 2 * D:3 * D])
                wv_stgs.append(stg)

            for c in range(2, 6):
                nc.sync.dma_start(x6[0][:, D * c:D * (c + 1)],
                                  x_il[0][:, D * c:D * (c + 1)])

            wj_stgs = []
            for d in range(6):
                for j in (0, 6):
                    stg = sb.tile([P, P], F32, tag="wjstage", bufs=3,
                                  name="wjstg")
                    nc.sync.dma_start(
                        stg[:], wqkv_d[P * d:P * (d + 1),
                                       P * j:P * (j + 1)])
                    wj_stgs.append(stg)

            for c in range(6, 8):
                nc.sync.dma_start(x6[0][:, D * c:D * (c + 1)],
                                  x_il[0][:, D * c:D * (c + 1)])

            # bias + bproj staging on the scalar DMA queue
            bstg = sb.tile([18, P], F32, tag="bstg", bufs=1, name="bstg")
            nc.scalar.dma_start(bstg[:], bqkv_d.rearrange("(j p) -> j p", p=P))
            bproj_row = sb.tile([1, D], F32, tag="bproj_row", bufs=1,
                                name="bproj_row")
            nc.scalar.dma_start(bproj_row[:], bproj_d.unsqueeze(0))

            # ---------- PE warm-up: ~4us of dummy matmuls flips HAM to 8/8
            # (transposes run in transpose-mode, which does not warm HAM)
            warm_h = sb.tile([P, 512], F16, tag="e", bufs=3, name="warm_h")
            nc.vector.memset(warm_h[:], 0.0)
            for wi in range(10):
                wps = ps.tile([P, 512], F32, tag="mm", bufs=2, name="wps")
                nc.tensor.matmul(wps[:], ones_h[:, 0:P], warm_h[:],
                                 start=True, stop=True)

            # ---------- W_qkv casts: V cols + j0/j6 on the (startup-idle)
            # scalar engine; the rest on DVE later (emission-order note:
            # these reads must precede the wrest DMAs' tag reuse).
            wq_h = []
            for d in range(6):
                t = sb.tile([P, D3], F16, tag=f"wqkv{d}", bufs=1,
                            name=f"wqkv{d}")
                nc.scalar.copy(t[:, 2 * D:3 * D], wv_stgs[d][:])
                for ji, j in enumerate((0, 6)):
                    nc.scalar.copy(t[:, P * j:P * (j + 1)],
                                   wj_stgs[2 * d + ji][:])
                wq_h.append(t)

            # remaining W_qkv columns (lands late; cast on DVE at startup
            # end, consumed by pair-1..5 QKV groups inside b0's window)
            wr_stgs = []
            for d in range(6):
                for rg in range(2):
                    c0 = (P, 7 * P)[rg]
                    stg = sb.tile([P, 5 * P], F32, tag="wstage", bufs=2,
                                  name="wrstg")
                    nc.sync.dma_start(
                        stg[:], wqkv_d[P * d:P * (d + 1), c0:c0 + 5 * P])
                    wr_stgs.append(stg)

            # ---------- bias pipeline (DVE + one PE transpose) ----------
            bstg_h = sb.tile([18, P], F16, tag="bstg_h", bufs=1, name="bstg_h")
            nc.vector.tensor_copy(bstg_h[:], bstg[:])
            btp = ps.tile([P, 18], F16, tag="mm", bufs=2, name="btp")
            nc.tensor.transpose(btp[:], bstg_h[:], ident[0:18, 0:18])
            bqkvT = sb.tile([P, 18], F32, tag="bqkvT", bufs=1, name="bqkvT")
            nc.vector.tensor_copy(bqkvT[:], btp[:])
            bv_h = sb.tile([P, 6], F16, tag="bv_h", bufs=1, name="bv_h")
            nc.vector.tensor_copy(bv_h[:], btp[:, 12:18])

            xT = {b: [sb.tile([P, N], F16, tag=f"xT{b}_{j}", bufs=1,
                              name=f"xT{b}_{j}") for j in range(6)]
                  for b in range(BL)}

            def emit_transpose_chunk(b, t):
                xh = sb.tile([P, D], F16, tag="xh", bufs=1, name="xh")
                nc.vector.tensor_copy(xh[:], x6[b][:, D * t:D * (t + 1)])
                for j in range(6):
                    tp = ps.tile([P, P], F16, tag="mm", bufs=2, name="tp")
                    nc.tensor.transpose(tp[:], xh[:, P * j:P * (j + 1)],
                                        ident[:])
                    nc.vector.tensor_copy(xT[b][j][:, P * t:P * (t + 1)],
                                          tp[:])

            # ---------- result tiles ----------
            qk = {b: [sb.tile([P, N], F16, tag=f"qk{b % 2}_{j}", bufs=1,
                              name=f"qk{j}") for j in range(12)]
                  for b in range(BL)}
            v = {b: [sb.tile([P, 12 * 65], F16, tag=f"v{b % 2}_{t}", bufs=1,
                             name=f"v{t}") for t in range(8)]
                 for b in range(BL)}
            at = {b: [sb.tile([P, N], F16, tag=f"at{j}", bufs=1,
                              name=f"at{j}") for j in range(6)]
                  for b in range(BL)}

            def emit_qkv_group(b, j, nh):
                qps = ps.tile([P, 512], F32, tag="mm", bufs=2, name="qps")
                for d in range(6):
                    nc.tensor.matmul(qps[:], wq_h[d][:, P * j:P * (j + 1)],
                                     xT[b][d][:, 512 * nh:512 * (nh + 1)],
                                     start=(d == 0), stop=(d == 5))
                nc.vector.tensor_scalar_add(
                    qk[b][j][:, 512 * nh:512 * (nh + 1)], qps[:],
                    bqkvT[:, j:j + 1])

            def emit_v_group(b, t, ci):
                c0, cw = ((0, 512), (512, 256))[ci]
                v3 = v[b][t].rearrange("p (h c) -> p h c", c=65)
                if ci == 0:
                    nc.vector.tensor_copy(v3[:, :, 64:65],
                                          ones_h[:, 0:12].unsqueeze(2))
                vps = ps.tile([P, 512], F32, tag="mm", bufs=2, name="vps")
                for d in range(6):
                    nc.tensor.matmul(vps[:, 0:cw],
                                     xT[b][d][:, P * t:P * (t + 1)],
                                     wq_h[d][:, 2 * D + c0:2 * D + c0 + cw],
                                     start=(d == 0), stop=(d == 5))
                nc.vector.tensor_copy(
                    v3[:, (c0 // HD):((c0 + cw) // HD), 0:HD],
                    vps[:, 0:cw].rearrange("p (h c) -> p h c", c=HD))

            # wp_h / bfinal state -- filled by fillers inside b0's window
            wp_h = []
            wpstgs = []
            bfb = sb.tile([P, D], F16, tag="bfb", bufs=1, name="bfb")

            def emit_wproj_dmas():
                for d in range(6):
                    stg = sb.tile([P, D], F32, tag="wstage", bufs=2,
                                  name="wpstg")
                    nc.scalar.dma_start(stg[:],
                                        wproj_d[P * d:P * (d + 1), :])
                    wpstgs.append(stg)

            def emit_wproj_cast(d):
                t = sb.tile([P, D], F16, tag=f"wproj{d}", bufs=1,
                            name=f"wproj{d}")
                nc.vector.tensor_copy(t[:], wpstgs[d][:])
                wp_h.append(t)

            def emit_bfinal():
                # b_final = W_proj^T b_v + b_proj  [1, 768] -> broadcast
                # to [128, 768] f16 via a DRAM bounce so the proj-output
                # copy can add it on the DVE (replaces 32 K=1 PE matmuls).
                bfinal_f = sb.tile([1, D], F32, tag="bfinal", bufs=1,
                                   name="bfinal")
                for c0, cw in ((0, 512), (512, 256)):
                    bf_ps = ps.tile([1, 512], F32, tag="mm", bufs=2,
                                    name="bf_ps")
                    for d in range(6):
                        nc.tensor.matmul(bf_ps[:, 0:cw], bv_h[:, d:d + 1],
                                         wp_h[d][:, c0:c0 + cw],
                                         start=(d == 0), stop=(d == 5))
                    nc.vector.tensor_add(bfinal_f[:, c0:c0 + cw],
                                         bf_ps[0:1, 0:cw],
                                         bproj_row[:, c0:c0 + cw])
                bfinal_h = sb.tile([1, D], F16, tag="bfinal_h", bufs=1,
                                   name="bfinal_h")
                nc.vector.tensor_copy(bfinal_h[:], bfinal_f[:])
                bf_d = dp.tile([1, D], F16, tag="bf_d", bufs=1, name="bf_d")
                nc.sync.dma_start(bf_d[:], bfinal_h[:])
                nc.sync.dma_start(bfb[:], bf_d[:].to_broadcast((P, D)))

            def emit_proj_half(b, t, ci):
                c0, cw = ((0, 512), (512, 256))[ci]
                pps = ps.tile([P, 512], F32, tag="mm", bufs=2, name="pps")
                for d in range(6):
                    nc.tensor.matmul(pps[:, 0:cw],
                                     at[b][d][:, P * t:P * (t + 1)],
                                     wp_h[d][:, c0:c0 + cw],
                                     start=(d == 0), stop=(d == 5))
                osb = sb.tile([P, 512], F32, tag="outs", bufs=2, name="osb")
                nc.vector.tensor_add(osb[:, 0:cw], pps[:, 0:cw],
                                     bfb[:, c0:c0 + cw])
                nc.sync.dma_start(out_il[b, t][:, c0:c0 + cw], osb[:, 0:cw])

            def pop_fillers(fillers, k=2):
                n = 0
                while fillers and n < k:
                    fillers.pop(0)()
                    n += 1

            # one-unit-deep queue of deferred normalize multiplies: the
            # reciprocal-broadcast DMA round trip gets a full unit of
            # latency slack instead of blocking the DVE stream in-line.
            pend_muls = []

            def flush_muls():
                while pend_muls:
                    pend_muls.pop(0)()

            def emit_unit(b, jp, nh, fillers, popk=2, tail=False):
                qt, kt = qk[b][jp], qk[b][6 + jp]
                n0 = 512 * nh
                ot = [ps.tile([65, 512], F32, tag="ot", bufs=2,
                              name="otps") for _ in range(2)]
                def emit_avs(pend):
                    for pm, pe_ in pend:
                        for hh in range(2):
                            h = 2 * jp + hh
                            nc.tensor.matmul(
                                ot[hh][:],
                                v[b][pm][:, 65 * h:65 * h + 65],
                                pe_[:, 512 * hh:512 * (hh + 1)],
                                start=(pm == 0), stop=(pm == 7))

                # m-blocks processed in pairs: the two score matmul pairs
                # are emitted back-to-back so the second pair's row-tiled
                # LDWEIGHTS pulls ahead behind a score MM (disjoint row
                # groups) instead of serializing behind a full-row AV MM.
                # AVs for the previous pair go before this pair's exps so
                # the e-buffer (bufs=3) WAR stays in emission order.
                pend = []
                for mp in range(4):
                    spss = []
                    for mi in range(2):
                        m = 2 * mp + mi
                        sps = ps.tile([P, N], F32, tag="s", bufs=2,
                                      name="sps")
                        for hh in range(2):
                            r0, r1 = HD * hh, HD * (hh + 1)
                            nc.tensor.matmul(
                                sps[:, 512 * hh:512 * (hh + 1)],
                                kt[r0:r1, P * m:P * (m + 1)],
                                qt[r0:r1, n0:n0 + 512],
                                start=True, stop=True)
                        spss.append((m, sps))
                    emit_avs(pend)
                    pend = []
                    for m, sps in spss:
                        e = sb.tile([P, N], F16, tag="e", bufs=3, name="e")
                        nc.scalar.activation(e[:], sps[:], AF.Exp,
                                             scale=SCALE)
                        pend.append((m, e))
                    if mp in (1, 2):
                        pop_fillers(fillers, popk)
                emit_avs(pend)
                # end-of-unit fillers come BEFORE the serial normalize
                # chain so their PSUM-group-closing DVE ops aren't queued
                # behind it (the next unit's filler matmuls WAR on them).
                pop_fillers(fillers, popk)
                # flush the PREVIOUS unit's deferred muls first: their bc
                # broadcasts landed a unit ago, and the bc/u_sb buffers
                # (bufs=2) are WAR-reused by this unit right after.
                flush_muls()

                def recip_chain(hh):
                    dr_f = sb.tile([1, 512], F32, tag="dr_f", bufs=1,
                                   name="dr_f")
                    nc.vector.tensor_copy(dr_f[:], ot[hh][64:65, :])
                    rr_f = sb.tile([1, 512], F32, tag="rr_f", bufs=1,
                                   name="rr_f")
                    nc.vector.reciprocal_approx_fast(out=rr_f[:],
                                                     in_=dr_f[:])
                    rr_h = sb.tile([1, 512], F16, tag="rr_h", bufs=1,
                                   name="rr_h")
                    nc.vector.tensor_copy(rr_h[:], rr_f[:])
                    rr_d = dp.tile([1, 512], F16, tag="rr_d", bufs=4,
                                   name="rr_d")
                    nc.sync.dma_start(rr_d[:], rr_h[:])
                    bc_h = sb.tile([HD, 512], F16, tag="bc_h", bufs=2,
                                   name="bc_h")
                    nc.sync.dma_start(bc_h[:],
                                      rr_d[:].to_broadcast((HD, 512)))
                    return bc_h

                def u_copy(hh):
                    # copy O^T out of PSUM promptly -- ot has no cross-
                    # unit slack (2 allocs/unit, bufs=2), so a deferred
                    # PSUM read would stall the next unit's first AV.
                    u_sb = sb.tile([HD, 512], F16, tag="u_sb", bufs=2,
                                   name="u_sb")
                    nc.vector.tensor_copy(u_sb[:], ot[hh][0:HD, :])
                    return u_sb

                if tail:
                    # window-final unit: the tail/bridge work waits on
                    # these muls, so skip the DMA broadcast round trip --
                    # broadcast the reciprocal row with a K=1 matmul on
                    # the idle-at-tail PE into an "s"-tag PSUM bank (free
                    # here; "mm" would WAR the tail projection groups)
                    # and multiply immediately.
                    for hh in range(2):
                        r0, r1 = HD * hh, HD * (hh + 1)
                        dr_f = sb.tile([1, 512], F32, tag="dr_f", bufs=1,
                                       name="dr_f")
                        nc.vector.tensor_copy(dr_f[:], ot[hh][64:65, :])
                        rr_f = sb.tile([1, 512], F32, tag="rr_f", bufs=1,
                                       name="rr_f")
                        nc.vector.reciprocal_approx_fast(out=rr_f[:],
                                                         in_=dr_f[:])
                        rr_h = sb.tile([1, 512], F16, tag="rr_h", bufs=1,
                                       name="rr_h")
                        nc.vector.tensor_copy(rr_h[:], rr_f[:])
                        bc_ps = ps.tile([P, N], F32, tag="s", bufs=2,
                                        name="bc_ps")
                        nc.tensor.matmul(bc_ps[0:HD, 0:512],
                                         ones_h[0:1, 0:HD], rr_h[:],
                                         start=True, stop=True)
                        u_sb = u_copy(hh)
                        nc.vector.tensor_mul(
                            at[b][jp][r0:r1, n0:n0 + 512],
                            u_sb[:], bc_ps[0:HD, 0:512])
                else:
                    us, bcs = [], []
                    for hh in range(2):
                        us.append(u_copy(hh))
                        bcs.append(recip_chain(hh))
                    for hh in range(2):
                        r0, r1 = HD * hh, HD * (hh + 1)
                        pend_muls.append(
                            lambda u=us[hh], bc=bcs[hh], r0=r0, r1=r1:
                            nc.vector.tensor_mul(
                                at[b][jp][r0:r1, n0:n0 + 512],
                                u[:], bc[:]))

            # ---------- startup: b0 transposes + QKV pair 0 + V(0,1);
            # V(2..7) ride as unit-0 fillers (popk=4 keeps each V(t)
            # ahead of the AV matmul that consumes it)
            for t in range(8):
                emit_transpose_chunk(0, t)
            for t in range(8):
                emit_v_group(0, t, 0)
                emit_v_group(0, t, 1)
            emit_qkv_group(0, 0, 0)
            emit_qkv_group(0, 6, 0)
            emit_qkv_group(0, 6, 1)

            # cast the late W_qkv columns on DVE (reads must be emitted
            # before the wproj DMAs reuse the wstage buffers)
            for d in range(6):
                for rg in range(2):
                    c0 = (P, 7 * P)[rg]
                    nc.vector.tensor_copy(wq_h[d][:, c0:c0 + 5 * P],
                                          wr_stgs[2 * d + rg][:])

            # issue remaining bulk DMAs now (land well before their use)
            emit_wproj_dmas()
            x6[1] = sb.tile([P, 8 * D], F32, tag="x6", bufs=1, name="x6")
            for c in range(8):
                nc.sync.dma_start(x6[1][:, D * c:D * (c + 1)],
                                  x_il[1][:, D * c:D * (c + 1)])

            # ---------- b0 attention, nh-outer ----------
            # nh0 fillers: pair jp+1's q/k groups front-loaded so each
            # pair's inputs are emitted a unit ahead of use (per-engine
            # streams are in-order: a consumer emitted before its producer
            # would deadlock), then q(*,nh1) groups, wproj casts, bfinal.
            q_nh0 = []
            for jpn in range(1, 6):
                q_nh0 += [
                    lambda j=jpn: emit_qkv_group(0, j, 0),
                    lambda j=6 + jpn: emit_qkv_group(0, j, 0),
                    lambda j=6 + jpn: emit_qkv_group(0, j, 1),
                    lambda j=jpn - 1: emit_qkv_group(0, j, 1),
                ]
            q_nh0 += [lambda: emit_qkv_group(0, 5, 1)]
            q_nh0 += [lambda d=d: emit_wproj_cast(d) for d in range(6)]
            q_nh0 += [emit_bfinal]
            for jp in range(6):
                emit_unit(0, jp, 0, q_nh0, tail=(jp == 5))
            flush_muls()
            while q_nh0:
                q_nh0.pop(0)()

            # nh1 fillers: b1 transposes + V(1) interleaved per chunk,
            # b1 QKV pair 0, then b0 proj for nh0 token chunks (0-3).
            q_nh1 = []
            for t in range(8):
                q_nh1 += [lambda t=t: emit_transpose_chunk(1, t)]
                q_nh1 += [lambda t=t, ci=ci: emit_v_group(1, t, ci)
                          for ci in range(2)]
            q_nh1 += [lambda: emit_qkv_group(1, 0, 0),
                      lambda: emit_qkv_group(1, 6, 0),
                      lambda: emit_qkv_group(1, 6, 1)]
            q_nh1 += [lambda t=t, ci=ci: emit_proj_half(0, t, ci)
                      for t in range(4) for ci in range(2)]
            for jp in range(6):
                emit_unit(0, jp, 1, q_nh1, tail=(jp == 5))
            flush_muls()
            while q_nh1:
                q_nh1.pop(0)()

            # ---------- b1 attention, nh-outer ----------
            # nh0 fillers: b1 QKV pairs 1-5 staged one unit ahead, then b0
            # proj for nh1 token chunks (popped at units 2+, after which
            # the at[0] reads are done so b1's at writes don't stall).
            q1_nh0 = [lambda: emit_qkv_group(1, 0, 1)]
            for jpn in range(1, 6):
                q1_nh0 += [
                    lambda j=jpn: emit_qkv_group(1, j, 0),
                    lambda j=6 + jpn: emit_qkv_group(1, j, 0),
                    lambda j=6 + jpn: emit_qkv_group(1, j, 1),
                    lambda j=jpn: emit_qkv_group(1, j, 1),
                ]
            q1_nh0 += [lambda t=t, ci=ci: emit_proj_half(0, t, ci)
                       for t in range(4, 8) for ci in range(2)]
            for jp in range(6):
                emit_unit(1, jp, 0, q1_nh0, tail=(jp == 5))
            flush_muls()
            while q1_nh0:
                q1_nh0.pop(0)()

            # nh1 fillers: b1 proj for nh0 token chunks (0-3).
            q1_nh1 = [lambda t=t, ci=ci: emit_proj_half(1, t, ci)
                      for t in range(4) for ci in range(2)]
            for jp in range(6):
                emit_unit(1, jp, 1, q1_nh1, tail=(jp == 5))
            flush_muls()
            while q1_nh1:
                q1_nh1.pop(0)()

            # ---------- tail: b1 proj for nh1 token chunks ----------
            for t in range(4, 8):
                for ci in range(2):
                    emit_proj_half(1, t, ci)
    nc.compile()
    return nc


def _get_nc():
    if "nc" not in _CACHE:
        _CACHE["nc"] = _build()
    return _CACHE["nc"]


def kernel(x, W_qkv, b_qkv, W_proj, b_proj):
    from concourse.bass_utils import run_bass_kernel_spmd

    nc = _get_nc()
    x = np.ascontiguousarray(x, dtype=np.float32)
    in_maps = [
        {
            "x": x[2 * i:2 * i + 2],
            "W_qkv": np.asarray(W_qkv, dtype=np.float32),
            "b_qkv": np.asarray(b_qkv, dtype=np.float32),
            "W_proj": np.asarray(W_proj, dtype=np.float32),
            "b_proj": np.asarray(b_proj, dtype=np.float32),
        }
        for i in range(8)
    ]
    res = run_bass_kernel_spmd(nc, in_maps, core_ids=list(range(8)))
    return np.concatenate([r["out"] for r in res.results], axis=0)


# revision 47
# speedup vs baseline: 1.0416x; 1.0217x over previous
"""Multi-head self-attention (B=16, N=1024, D=768, H=12) on 8 TRN2 NeuronCores.

Data-parallel over batch (2 batches per core, weights replicated, no
collectives). Per core, one fused Bass/Tile kernel:

  x --8 chunked interleaved DMAs--> x6 [128, 8*768] (token 8p+t on
      partition p, slot t; attention is permutation-invariant over tokens,
      so the interleave is only undone at the output DMA)
  x6 --f16 cast + PE transpose--> xT [d, tok]
  QT/KT = (W_qkv^T x^T + b) in [col, tok] layout (f16)
  V_aug = [x W_v | ones-col per head]  [tok, 12*65] (f16)
  per head: S^T[m,n] = K Q^T (PE), E = exp(S^T*scale) (ACT, [128,1024]),
      O^T = V_aug^T E (PE; row 64 = softmax denominator via the ones
      column -- no max subtraction needed, scores are O(1)).
      normalize: recip_approx_fast straight from PSUM row 64,
      DMA-broadcast the reciprocal row (f32), one DVE mul reading PSUM.
  out = attnT^T W_proj, bias (W_proj^T b_v + b_proj) added by the DVE
      during the PSUM->SBUF copy from a DMA-broadcast bias tile
      (V-bias folded through softmax since rows of A sum to 1)

All matmul operands f16 (1 cycle/row; fp32/f32r run 2-pass fp32_mode=HIGH
at 1/4 rate and break HAM warm-up -- measured). PSUM accumulation is f32.

Scheduling: engines execute fixed in-order streams, so emission order IS
the schedule. Startup pipelines chunked x DMAs -> transposes -> V/QKV so
batch-0 attention starts ~30us earlier than a phase-serial ordering.
Attention loops are nh-outer so each batch's first-half projection becomes
filler work inside its own attention window; b1's QKV pairs 1-5 are
deferred into b1's window to keep the PE fed there (stalled PE breaks the
LDWEIGHTS pull-ahead and inflates score MMs from 213ns to 318ns).
"""

import numpy as np

_CACHE: dict = {}

P = 128
BL, N, D, H, HD = 2, 1024, 768, 12, 64
D3 = 3 * D
SCALE = float(HD) ** -0.5


def _build():
    import concourse.mybir as mybir
    import concourse.tile as tile
    from concourse import bacc
    from concourse.masks import make_identity

    dt = mybir.dt
    F32, F16 = dt.float32, dt.float16
    AF = mybir.ActivationFunctionType

    nc = bacc.Bacc("TRN2", target_bir_lowering=False, debug=False)
    x_d = nc.dram_tensor("x", [BL, N, D], F32, kind="ExternalInput").ap()
    wqkv_d = nc.dram_tensor("W_qkv", [D, D3], F32, kind="ExternalInput").ap()
    bqkv_d = nc.dram_tensor("b_qkv", [D3], F32, kind="ExternalInput").ap()
    wproj_d = nc.dram_tensor("W_proj", [D, D], F32, kind="ExternalInput").ap()
    bproj_d = nc.dram_tensor("b_proj", [D], F32, kind="ExternalInput").ap()
    out_d = nc.dram_tensor("out", [BL, N, D], F32, kind="ExternalOutput").ap()
    # token-interleaved views: partition p, slot t <-> token 8p+t
    x_il = x_d.rearrange("b (p i) d -> b p (i d)", p=P)       # [2, 128, 6144]
    out_il = out_d.rearrange("b (p i) d -> b i p d", p=P)     # [2, 8, 128, 768]

    with tile.TileContext(nc) as tc:
        with tc.tile_pool(name="sb", bufs=1) as sb, \
             tc.tile_pool(name="dp", bufs=1, space="DRAM") as dp, \
             tc.tile_pool(name="ps", bufs=2, space="PSUM") as ps:

            # ---------- constants ----------
            ident = sb.tile([P, P], F16, tag="ident", bufs=1, name="ident")
            make_identity(nc, ident[:])
            ones_h = sb.tile([P, P], F16, tag="ones_h", bufs=1, name="ones_h")
            nc.vector.memset(ones_h[:], 1.0)

            # ---------- DMA issue order, by first use: V columns of W_qkv,
            # x chunks 0-3, the j=0/j=6 W columns (QKV pair 0), x chunks
            # 4-7, then the remaining W columns (pairs 1-5, used as b0-
            # window fillers so they may land late).
            x6 = {}
            x6[0] = sb.tile([P, 8 * D], F32, tag="x6", bufs=1, name="x6")
            for c in range(2):
                nc.sync.dma_start(x6[0][:, D * c:D * (c + 1)],
                                  x_il[0][:, D * c:D * (c + 1)])

            wv_stgs = []
            for d in range(6):
                stg = sb.tile([P, D], F32, tag="wstage", bufs=2,
                              name="wvstg")
                nc.sync.dma_start(stg[:],
                                  wqkv_d[P * d:P * (d + 1), 2 * D:3 * D])
                wv_stgs.append(stg)

            for c in range(2, 6):
                nc.sync.dma_start(x6[0][:, D * c:D * (c + 1)],
                                  x_il[0][:, D * c:D * (c + 1)])

            wj_stgs = []
            for d in range(6):
                for j in (0, 6):
                    stg = sb.tile([P, P], F32, tag="wjstage", bufs=3,
                                  name="wjstg")
                    nc.sync.dma_start(
                        stg[:], wqkv_d[P * d:P * (d + 1),
                                       P * j:P * (j + 1)])
                    wj_stgs.append(stg)

            for c in range(6, 8):
                nc.sync.dma_start(x6[0][:, D * c:D * (c + 1)],
                                  x_il[0][:, D * c:D * (c + 1)])

            # bias + bproj staging on the scalar DMA queue
            bstg = sb.tile([18, P], F32, tag="bstg", bufs=1, name="bstg")
            nc.scalar.dma_start(bstg[:], bqkv_d.rearrange("(j p) -> j p", p=P))
            bproj_row = sb.tile([1, D], F32, tag="bproj_row", bufs=1,
                                name="bproj_row")
            nc.scalar.dma_start(bproj_row[:], bproj_d.unsqueeze(0))

            # ---------- PE warm-up: ~4us of dummy matmuls flips HAM to 8/8
            # (transposes run in transpose-mode, which does not warm HAM)
            warm_h = sb.tile([P, 512], F16, tag="e", bufs=3, name="warm_h")
            nc.vector.memset(warm_h[:], 0.0)
            for wi in range(10):
                wps = ps.tile([P, 512], F32, tag="mm", bufs=2, name="wps")
                nc.tensor.matmul(wps[:], ones_h[:, 0:P], warm_h[:],
                                 start=True, stop=True)

            # ---------- W_qkv casts: V cols + j0/j6 on the (startup-idle)
            # scalar engine; the rest on DVE later (emission-order note:
            # these reads must precede the wrest DMAs' tag reuse).
            wq_h = []
            for d in range(6):
                t = sb.tile([P, D3], F16, tag=f"wqkv{d}", bufs=1,
                            name=f"wqkv{d}")
                nc.scalar.copy(t[:, 2 * D:3 * D], wv_stgs[d][:])
                for ji, j in enumerate((0, 6)):
                    nc.scalar.copy(t[:, P * j:P * (j + 1)],
                                   wj_stgs[2 * d + ji][:])
                wq_h.append(t)

            # remaining W_qkv columns (lands late; cast on DVE at startup
            # end, consumed by pair-1..5 QKV groups inside b0's window)
            wr_stgs = []
            for d in range(6):
                for rg in range(2):
                    c0 = (P, 7 * P)[rg]
                    stg = sb.tile([P, 5 * P], F32, tag="wstage", bufs=2,
                                  name="wrstg")
                    nc.sync.dma_start(
                        stg[:], wqkv_d[P * d:P * (d + 1), c0:c0 + 5 * P])
                    wr_stgs.append(stg)

            # ---------- bias pipeline (DVE + one PE transpose) ----------
            bstg_h = sb.tile([18, P], F16, tag="bstg_h", bufs=1, name="bstg_h")
            nc.vector.tensor_copy(bstg_h[:], bstg[:])
            btp = ps.tile([P, 18], F16, tag="mm", bufs=2, name="btp")
            nc.tensor.transpose(btp[:], bstg_h[:], ident[0:18, 0:18])
            bqkvT = sb.tile([P, 18], F32, tag="bqkvT", bufs=1, name="bqkvT")
            nc.vector.tensor_copy(bqkvT[:], btp[:])
            bv_h = sb.tile([P, 6], F16, tag="bv_h", bufs=1, name="bv_h")
            nc.vector.tensor_copy(bv_h[:], btp[:, 12:18])

            xT = {b: [sb.tile([P, N], F16, tag=f"xT{b}_{j}", bufs=1,
                              name=f"xT{b}_{j}") for j in range(6)]
                  for b in range(BL)}

            def emit_transpose_chunk(b, t):
                xh = sb.tile([P, D], F16, tag="xh", bufs=1, name="xh")
                nc.vector.tensor_copy(xh[:], x6[b][:, D * t:D * (t + 1)])
                for j in range(6):
                    tp = ps.tile([P, P], F16, tag="mm", bufs=2, name="tp")
                    nc.tensor.transpose(tp[:], xh[:, P * j:P * (j + 1)],
                                        ident[:])
                    nc.vector.tensor_copy(xT[b][j][:, P * t:P * (t + 1)],
                                          tp[:])

            # ---------- result tiles ----------
            qk = {b: [sb.tile([P, N], F16, tag=f"qk{b % 2}_{j}", bufs=1,
                              name=f"qk{j}") for j in range(12)]
                  for b in range(BL)}
            v = {b: [sb.tile([P, 12 * 65], F16, tag=f"v{b % 2}_{t}", bufs=1,
                             name=f"v{t}") for t in range(8)]
                 for b in range(BL)}
            at = {b: [sb.tile([P, N], F16, tag=f"at{j}", bufs=1,
                              name=f"at{j}") for j in range(6)]
                  for b in range(BL)}

            def emit_qkv_group(b, j, nh):
                qps = ps.tile([P, 512], F32, tag="mm", bufs=2, name="qps")
                for d in range(6):
                    nc.tensor.matmul(qps[:], wq_h[d][:, P * j:P * (j + 1)],
                                     xT[b][d][:, 512 * nh:512 * (nh + 1)],
                                     start=(d == 0), stop=(d == 5))
                nc.vector.tensor_scalar_add(
                    qk[b][j][:, 512 * nh:512 * (nh + 1)], qps[:],
                    bqkvT[:, j:j + 1])

            def emit_v_group(b, t, ci):
                c0, cw = ((0, 512), (512, 256))[ci]
                v3 = v[b][t].rearrange("p (h c) -> p h c", c=65)
                if ci == 0:
                    nc.vector.tensor_copy(v3[:, :, 64:65],
                                          ones_h[:, 0:12].unsqueeze(2))
                vps = ps.tile([P, 512], F32, tag="mm", bufs=2, name="vps")
                for d in range(6):
                    nc.tensor.matmul(vps[:, 0:cw],
                                     xT[b][d][:, P * t:P * (t + 1)],
                                     wq_h[d][:, 2 * D + c0:2 * D + c0 + cw],
                                     start=(d == 0), stop=(d == 5))
                nc.vector.tensor_copy(
                    v3[:, (c0 // HD):((c0 + cw) // HD), 0:HD],
                    vps[:, 0:cw].rearrange("p (h c) -> p h c", c=HD))

            # wp_h / bfinal state -- filled by fillers inside b0's window
            wp_h = []
            wpstgs = []
            bfb = sb.tile([P, D], F16, tag="bfb", bufs=1, name="bfb")

            def emit_wproj_dmas():
                for d in range(6):
                    stg = sb.tile([P, D], F32, tag="wstage", bufs=2,
                                  name="wpstg")
                    nc.scalar.dma_start(stg[:],
                                        wproj_d[P * d:P * (d + 1), :])
                    wpstgs.append(stg)

            def emit_wproj_cast(d):
                t = sb.tile([P, D], F16, tag=f"wproj{d}", bufs=1,
                            name=f"wproj{d}")
                nc.vector.tensor_copy(t[:], wpstgs[d][:])
                wp_h.append(t)

            def emit_bfinal():
                # b_final = W_proj^T b_v + b_proj  [1, 768] -> broadcast
                # to [128, 768] f16 via a DRAM bounce so the proj-output
                # copy can add it on the DVE (replaces 32 K=1 PE matmuls).
                bfinal_f = sb.tile([1, D], F32, tag="bfinal", bufs=1,
                                   name="bfinal")
                for c0, cw in ((0, 512), (512, 256)):
                    bf_ps = ps.tile([1, 512], F32, tag="mm", bufs=2,
                                    name="bf_ps")
                    for d in range(6):
                        nc.tensor.matmul(bf_ps[:, 0:cw], bv_h[:, d:d + 1],
                                         wp_h[d][:, c0:c0 + cw],
                                         start=(d == 0), stop=(d == 5))
                    nc.vector.tensor_add(bfinal_f[:, c0:c0 + cw],
                                         bf_ps[0:1, 0:cw],
                                         bproj_row[:, c0:c0 + cw])
                bfinal_h = sb.tile([1, D], F16, tag="bfinal_h", bufs=1,
                                   name="bfinal_h")
                nc.vector.tensor_copy(bfinal_h[:], bfinal_f[:])
                bf_d = dp.tile([1, D], F16, tag="bf_d", bufs=1, name="bf_d")
                nc.sync.dma_start(bf_d[:], bfinal_h[:])
                nc.sync.dma_start(bfb[:], bf_d[:].to_broadcast((P, D)))

            def emit_proj_half(b, t, ci):
                c0, cw = ((0, 512), (512, 256))[ci]
                pps = ps.tile([P, 512], F32, tag="mm", bufs=2, name="pps")
                for d in range(6):
                    nc.tensor.matmul(pps[:, 0:cw],
                                     at[b][d][:, P * t:P * (t + 1)],
                                     wp_h[d][:, c0:c0 + cw],
                                     start=(d == 0), stop=(d == 5))
                osb = sb.tile([P, 512], F32, tag="outs", bufs=2, name="osb")
                nc.vector.tensor_add(osb[:, 0:cw], pps[:, 0:cw],
                                     bfb[:, c0:c0 + cw])
                nc.sync.dma_start(out_il[b, t][:, c0:c0 + cw], osb[:, 0:cw])

            def pop_fillers(fillers, k=2):
                n = 0
                while fillers and n < k:
                    fillers.pop(0)()
                    n += 1

            # one-unit-deep queue of deferred normalize multiplies: the
            # reciprocal-broadcast DMA round trip gets a full unit of
            # latency slack instead of blocking the DVE stream in-line.
            pend_muls = []

            def flush_muls():
                while pend_muls:
                    pend_muls.pop(0)()

            def emit_unit(b, jp, nh, fillers, popk=2, tail=False):
                qt, kt = qk[b][jp], qk[b][6 + jp]
                n0 = 512 * nh
                ot = [ps.tile([65, 512], F32, tag="ot", bufs=2,
                              name="otps") for _ in range(2)]
                def emit_avs(pend):
                    for pm, pe_ in pend:
                        for hh in range(2):
                            h = 2 * jp + hh
                            nc.tensor.matmul(
                                ot[hh][:],
                                v[b][pm][:, 65 * h:65 * h + 65],
                                pe_[:, 512 * hh:512 * (hh + 1)],
                                start=(pm == 0), stop=(pm == 7))

                # m-blocks processed in pairs: the two score matmul pairs
                # are emitted back-to-back so the second pair's row-tiled
                # LDWEIGHTS pulls ahead behind a score MM (disjoint row
                # groups) instead of serializing behind a full-row AV MM.
                # AVs for the previous pair go before this pair's exps so
                # the e-buffer (bufs=3) WAR stays in emission order.
                pend = []
                for mp in range(4):
                    spss = []
                    for mi in range(2):
                        m = 2 * mp + mi
                        sps = ps.tile([P, N], F32, tag="s", bufs=2,
                                      name="sps")
                        for hh in range(2):
                            r0, r1 = HD * hh, HD * (hh + 1)
                            nc.tensor.matmul(
                                sps[:, 512 * hh:512 * (hh + 1)],
                                kt[r0:r1, P * m:P * (m + 1)],
                                qt[r0:r1, n0:n0 + 512],
                                start=True, stop=True)
                        spss.append((m, sps))
                    emit_avs(pend)
                    pend = []
                    for m, sps in spss:
                        e = sb.tile([P, N], F16, tag="e", bufs=3, name="e")
                        nc.scalar.activation(e[:], sps[:], AF.Exp,
                                             scale=SCALE)
                        pend.append((m, e))
                    if mp in (1, 2):
                        pop_fillers(fillers, popk)
                emit_avs(pend)
                # end-of-unit fillers come BEFORE the serial normalize
                # chain so their PSUM-group-closing DVE ops aren't queued
                # behind it (the next unit's filler matmuls WAR on them).
                pop_fillers(fillers, popk)
                # flush the PREVIOUS unit's deferred muls first: their bc
                # broadcasts landed a unit ago, and the bc/u_sb buffers
                # (bufs=2) are WAR-reused by this unit right after.
                flush_muls()

                def recip_chain(hh):
                    dr_f = sb.tile([1, 512], F32, tag="dr_f", bufs=1,
                                   name="dr_f")
                    nc.vector.tensor_copy(dr_f[:], ot[hh][64:65, :])
                    rr_f = sb.tile([1, 512], F32, tag="rr_f", bufs=1,
                                   name="rr_f")
                    nc.vector.reciprocal_approx_fast(out=rr_f[:],
                                                     in_=dr_f[:])
                    rr_h = sb.tile([1, 512], F16, tag="rr_h", bufs=1,
                                   name="rr_h")
                    nc.vector.tensor_copy(rr_h[:], rr_f[:])
                    rr_d = dp.tile([1, 512], F16, tag="rr_d", bufs=4,
                                   name="rr_d")
                    nc.sync.dma_start(rr_d[:], rr_h[:])
                    bc_h = sb.tile([HD, 512], F16, tag="bc_h", bufs=2,
                                   name="bc_h")
                    nc.sync.dma_start(bc_h[:],
                                      rr_d[:].to_broadcast((HD, 512)))
                    return bc_h

                def u_copy(hh):
                    # copy O^T out of PSUM promptly -- ot has no cross-
                    # unit slack (2 allocs/unit, bufs=2), so a deferred
                    # PSUM read would stall the next unit's first AV.
                    u_sb = sb.tile([HD, 512], F16, tag="u_sb", bufs=2,
                                   name="u_sb")
                    nc.vector.tensor_copy(u_sb[:], ot[hh][0:HD, :])
                    return u_sb

                if tail:
                    # window-final unit: launch both broadcast round trips
                    # before the O^T copies -- the tail projection waits
                    # on the muls, so the DMA latency is critical here.
                    bcs = [recip_chain(0), recip_chain(1)]
                    us = [u_copy(0), u_copy(1)]
                else:
                    us, bcs = [], []
                    for hh in range(2):
                        us.append(u_copy(hh))
                        bcs.append(recip_chain(hh))
                for hh in range(2):
                    r0, r1 = HD * hh, HD * (hh + 1)
                    pend_muls.append(
                        lambda u=us[hh], bc=bcs[hh], r0=r0, r1=r1:
                        nc.vector.tensor_mul(at[b][jp][r0:r1, n0:n0 + 512],
                                             u[:], bc[:]))

            # ---------- startup: b0 transposes + QKV pair 0 + V(0,1);
            # V(2..7) ride as unit-0 fillers (popk=4 keeps each V(t)
            # ahead of the AV matmul that consumes it)
            for t in range(8):
                emit_transpose_chunk(0, t)
            for t in range(8):
                emit_v_group(0, t, 0)
                emit_v_group(0, t, 1)
            emit_qkv_group(0, 0, 0)
            emit_qkv_group(0, 6, 0)
            emit_qkv_group(0, 6, 1)

            # cast the late W_qkv columns on DVE (reads must be emitted
            # before the wproj DMAs reuse the wstage buffers)
            for d in range(6):
                for rg in range(2):
                    c0 = (P, 7 * P)[rg]
                    nc.vector.tensor_copy(wq_h[d][:, c0:c0 + 5 * P],
                                          wr_stgs[2 * d + rg][:])

            # issue remaining bulk DMAs now (land well before their use)
            emit_wproj_dmas()
            x6[1] = sb.tile([P, 8 * D], F32, tag="x6", bufs=1, name="x6")
            for c in range(8):
                nc.sync.dma_start(x6[1][:, D * c:D * (c + 1)],
                                  x_il[1][:, D * c:D * (c + 1)])

            # ---------- b0 attention, nh-outer ----------
            # nh0 fillers: pair jp+1's q/k groups front-loaded so each
            # pair's inputs are emitted a unit ahead of use (per-engine
            # streams are in-order: a consumer emitted before its producer
            # would deadlock), then q(*,nh1) groups, wproj casts, bfinal.
            q_nh0 = []
            for jpn in range(1, 6):
                q_nh0 += [
                    lambda j=jpn: emit_qkv_group(0, j, 0),
                    lambda j=6 + jpn: emit_qkv_group(0, j, 0),
                    lambda j=6 + jpn: emit_qkv_group(0, j, 1),
                    lambda j=jpn - 1: emit_qkv_group(0, j, 1),
                ]
            q_nh0 += [lambda: emit_qkv_group(0, 5, 1)]
            q_nh0 += [lambda d=d: emit_wproj_cast(d) for d in range(6)]
            q_nh0 += [emit_bfinal]
            for jp in range(6):
                emit_unit(0, jp, 0, q_nh0, tail=(jp == 5))
            flush_muls()
            while q_nh0:
                q_nh0.pop(0)()

            # nh1 fillers: b1 transposes + V(1) interleaved per chunk,
            # b1 QKV pair 0, then b0 proj for nh0 token chunks (0-3).
            q_nh1 = []
            for t in range(8):
                q_nh1 += [lambda t=t: emit_transpose_chunk(1, t)]
                q_nh1 += [lambda t=t, ci=ci: emit_v_group(1, t, ci)
                          for ci in range(2)]
            q_nh1 += [lambda: emit_qkv_group(1, 0, 0),
                      lambda: emit_qkv_group(1, 6, 0),
                      lambda: emit_qkv_group(1, 6, 1)]
            q_nh1 += [lambda t=t, ci=ci: emit_proj_half(0, t, ci)
                      for t in range(4) for ci in range(2)]
            for jp in range(6):
                emit_unit(0, jp, 1, q_nh1, tail=(jp == 5))
            flush_muls()
            while q_nh1:
                q_nh1.pop(0)()

            # ---------- b1 attention, nh-outer ----------
            # nh0 fillers: b1 QKV pairs 1-5 staged one unit ahead, then b0
            # proj for nh1 token chunks (popped at units 2+, after which
            # the at[0] reads are done so b1's at writes don't stall).
            q1_nh0 = [lambda: emit_qkv_group(1, 0, 1)]
            for jpn in range(1, 6):
                q1_nh0 += [
                    lambda j=jpn: emit_qkv_group(1, j, 0),
                    lambda j=6 + jpn: emit_qkv_group(1, j, 0),
                    lambda j=6 + jpn: emit_qkv_group(1, j, 1),
                    lambda j=jpn: emit_qkv_group(1, j, 1),
                ]
            q1_nh0 += [lambda t=t, ci=ci: emit_proj_half(0, t, ci)
                       for t in range(4, 8) for ci in range(2)]
            for jp in range(6):
                emit_unit(1, jp, 0, q1_nh0, tail=(jp == 5))
            flush_muls()
            while q1_nh0:
                q1_nh0.pop(0)()

            # nh1 fillers: b1 proj for nh0 token chunks (0-3).
            q1_nh1 = [lambda t=t, ci=ci: emit_proj_half(1, t, ci)
                      for t in range(4) for ci in range(2)]
            for jp in range(6):
                emit_unit(1, jp, 1, q1_nh1, tail=(jp == 5))
            flush_muls()
            while q1_nh1:
                q1_nh1.pop(0)()

            # ---------- tail: b1 proj for nh1 token chunks ----------
            for t in range(4, 8):
                for ci in range(2):
                    emit_proj_half(1, t, ci)
    nc.compile()
    return nc


def _get_nc():
    if "nc" not in _CACHE:
        _CACHE["nc"] = _build()
    return _CACHE["nc"]


def kernel(x, W_qkv, b_qkv, W_proj, b_proj):
    from concourse.bass_utils import run_bass_kernel_spmd

    nc = _get_nc()
    x = np.ascontiguousarray(x, dtype=np.float32)
    in_maps = [
        {
            "x": x[2 * i:2 * i + 2],
            "W_qkv": np.asarray(W_qkv, dtype=np.float32),
            "b_qkv": np.asarray(b_qkv, dtype=np.float32),
            "W_proj": np.asarray(W_proj, dtype=np.float32),
            "b_proj": np.asarray(b_proj, dtype=np.float32),
        }
        for i in range(8)
    ]
    res = run_bass_kernel_spmd(nc, in_maps, core_ids=list(range(8)))
    return np.concatenate([r["out"] for r in res.results], axis=0)


# revision 48
# speedup vs baseline: 1.0938x; 1.0502x over previous
"""Multi-head self-attention (B=16, N=1024, D=768, H=12) on 8 TRN2 NeuronCores.

Data-parallel over batch (2 batches per core, weights replicated, no
collectives). Per core, one fused Bass/Tile kernel:

  x --8 chunked interleaved DMAs--> x6 [128, 8*768] (token 8p+t on
      partition p, slot t; attention is permutation-invariant over tokens,
      so the interleave is only undone at the output DMA)
  x6 --f16 cast + PE transpose--> xT [d, tok]
  QT/KT = (W_qkv^T x^T + b) in [col, tok] layout (f16)
  V_aug = [x W_v | ones-col per head]  [tok, 12*65] (f16)
  per head: S^T[m,n] = K Q^T (PE), E = exp(S^T*scale) (ACT, [128,1024]),
      O^T = V_aug^T E (PE; row 64 = softmax denominator via the ones
      column -- no max subtraction needed, scores are O(1)).
      normalize: recip_approx_fast straight from PSUM row 64,
      DMA-broadcast the reciprocal row (f32), one DVE mul reading PSUM.
  out = attnT^T W_proj, bias (W_proj^T b_v + b_proj) added by the DVE
      during the PSUM->SBUF copy from a DMA-broadcast bias tile
      (V-bias folded through softmax since rows of A sum to 1)

All matmul operands f16 (1 cycle/row; fp32/f32r run 2-pass fp32_mode=HIGH
at 1/4 rate and break HAM warm-up -- measured). PSUM accumulation is f32.

Scheduling: engines execute fixed in-order streams, so emission order IS
the schedule. Startup pipelines chunked x DMAs -> transposes -> V/QKV so
batch-0 attention starts ~30us earlier than a phase-serial ordering.
Attention loops are nh-outer so each batch's first-half projection becomes
filler work inside its own attention window; b1's QKV pairs 1-5 are
deferred into b1's window to keep the PE fed there (stalled PE breaks the
LDWEIGHTS pull-ahead and inflates score MMs from 213ns to 318ns).
"""

import numpy as np

_CACHE: dict = {}

P = 128
BL, N, D, H, HD = 2, 1024, 768, 12, 64
D3 = 3 * D
SCALE = float(HD) ** -0.5


def _build():
    import concourse.mybir as mybir
    import concourse.tile as tile
    from concourse import bacc
    from concourse.masks import make_identity

    dt = mybir.dt
    F32, F16 = dt.float32, dt.float16
    AF = mybir.ActivationFunctionType

    nc = bacc.Bacc("TRN2", target_bir_lowering=False, debug=False)
    x_d = nc.dram_tensor("x", [BL, N, D], F32, kind="ExternalInput").ap()
    wqkv_d = nc.dram_tensor("W_qkv", [D, D3], F32, kind="ExternalInput").ap()
    bqkv_d = nc.dram_tensor("b_qkv", [D3], F32, kind="ExternalInput").ap()
    wproj_d = nc.dram_tensor("W_proj", [D, D], F32, kind="ExternalInput").ap()
    bproj_d = nc.dram_tensor("b_proj", [D], F32, kind="ExternalInput").ap()
    out_d = nc.dram_tensor("out", [BL, N, D], F32, kind="ExternalOutput").ap()
    # token-interleaved views: partition p, slot t <-> token 8p+t
    x_il = x_d.rearrange("b (p i) d -> b p (i d)", p=P)       # [2, 128, 6144]
    out_il = out_d.rearrange("b (p i) d -> b i p d", p=P)     # [2, 8, 128, 768]

    with tile.TileContext(nc) as tc:
        with tc.tile_pool(name="sb", bufs=1) as sb, \
             tc.tile_pool(name="dp", bufs=1, space="DRAM") as dp, \
             tc.tile_pool(name="ps", bufs=2, space="PSUM") as ps:

            # ---------- constants ----------
            ident = sb.tile([P, P], F16, tag="ident", bufs=1, name="ident")
            make_identity(nc, ident[:])
            ones_h = sb.tile([P, P], F16, tag="ones_h", bufs=1, name="ones_h")
            nc.vector.memset(ones_h[:], 1.0)

            # ---------- DMA issue order, by first use: V columns of W_qkv,
            # x chunks 0-3, the j=0/j=6 W columns (QKV pair 0), x chunks
            # 4-7, then the remaining W columns (pairs 1-5, used as b0-
            # window fillers so they may land late).
            # x staged per 768-col slot through a rotating 3-buffer pool
            # (saves 15KB/partition vs a monolithic [128, 6144] tile; the
            # freed space un-aliases the two batches' at tiles below).
            xstg = {}

            def emit_x_dma(b, t):
                stg = sb.tile([P, D], F32, tag="xstage", bufs=3,
                              name="xstg")
                nc.sync.dma_start(stg[:], x_il[b][:, D * t:D * (t + 1)])
                xstg[(b, t)] = stg

            for c in range(2):
                emit_x_dma(0, c)

            wv_stgs = []
            for d in range(6):
                stg = sb.tile([P, D], F32, tag="wstage", bufs=2,
                              name="wvstg")
                nc.sync.dma_start(stg[:],
                                  wqkv_d[P * d:P * (d + 1), 2 * D:3 * D])
                wv_stgs.append(stg)

            emit_x_dma(0, 2)

            wj_stgs = []
            for d in range(6):
                for j in (0, 6):
                    stg = sb.tile([P, P], F32, tag="wjstage", bufs=3,
                                  name="wjstg")
                    nc.sync.dma_start(
                        stg[:], wqkv_d[P * d:P * (d + 1),
                                       P * j:P * (j + 1)])
                    wj_stgs.append(stg)


            # bias + bproj staging on the scalar DMA queue
            bstg = sb.tile([18, P], F32, tag="bstg", bufs=1, name="bstg")
            nc.scalar.dma_start(bstg[:], bqkv_d.rearrange("(j p) -> j p", p=P))
            bproj_row = sb.tile([1, D], F32, tag="bproj_row", bufs=1,
                                name="bproj_row")
            nc.scalar.dma_start(bproj_row[:], bproj_d.unsqueeze(0))

            # ---------- PE warm-up: ~4us of dummy matmuls flips HAM to 8/8
            # (transposes run in transpose-mode, which does not warm HAM)
            warm_h = sb.tile([P, 512], F16, tag="e", bufs=3, name="warm_h")
            nc.vector.memset(warm_h[:], 0.0)
            for wi in range(10):
                wps = ps.tile([P, 512], F32, tag="mm", bufs=2, name="wps")
                nc.tensor.matmul(wps[:], ones_h[:, 0:P], warm_h[:],
                                 start=True, stop=True)

            # ---------- W_qkv casts: V cols + j0/j6 on the (startup-idle)
            # scalar engine; the rest on DVE later (emission-order note:
            # these reads must precede the wrest DMAs' tag reuse).
            wq_h = []
            for d in range(6):
                t = sb.tile([P, D3], F16, tag=f"wqkv{d}", bufs=1,
                            name=f"wqkv{d}")
                nc.scalar.copy(t[:, 2 * D:3 * D], wv_stgs[d][:])
                for ji, j in enumerate((0, 6)):
                    nc.scalar.copy(t[:, P * j:P * (j + 1)],
                                   wj_stgs[2 * d + ji][:])
                wq_h.append(t)

            # remaining W_qkv columns (lands late; cast on DVE at startup
            # end, consumed by pair-1..5 QKV groups inside b0's window)
            wr_stgs = []
            for d in range(6):
                for rg in range(2):
                    c0 = (P, 7 * P)[rg]
                    stg = sb.tile([P, 5 * P], F32, tag="wstage", bufs=2,
                                  name="wrstg")
                    nc.sync.dma_start(
                        stg[:], wqkv_d[P * d:P * (d + 1), c0:c0 + 5 * P])
                    wr_stgs.append(stg)

            # ---------- bias pipeline (DVE + one PE transpose) ----------
            bstg_h = sb.tile([18, P], F16, tag="bstg_h", bufs=1, name="bstg_h")
            nc.vector.tensor_copy(bstg_h[:], bstg[:])
            btp = ps.tile([P, 18], F16, tag="mm", bufs=2, name="btp")
            nc.tensor.transpose(btp[:], bstg_h[:], ident[0:18, 0:18])
            bqkvT = sb.tile([P, 18], F32, tag="bqkvT", bufs=1, name="bqkvT")
            nc.vector.tensor_copy(bqkvT[:], btp[:])
            bv_h = sb.tile([P, 6], F16, tag="bv_h", bufs=1, name="bv_h")
            nc.vector.tensor_copy(bv_h[:], btp[:, 12:18])

            xT = {b: [sb.tile([P, N], F16, tag=f"xT{b}_{j}", bufs=1,
                              name=f"xT{b}_{j}") for j in range(6)]
                  for b in range(BL)}

            def emit_transpose_chunk(b, t):
                xh = sb.tile([P, D], F16, tag="xh", bufs=1, name="xh")
                nc.vector.tensor_copy(xh[:], xstg[(b, t)][:])
                if t + 3 < 8:
                    emit_x_dma(b, t + 3)
                for j in range(6):
                    tp = ps.tile([P, P], F16, tag="mm", bufs=2, name="tp")
                    nc.tensor.transpose(tp[:], xh[:, P * j:P * (j + 1)],
                                        ident[:])
                    nc.vector.tensor_copy(xT[b][j][:, P * t:P * (t + 1)],
                                          tp[:])

            # ---------- result tiles ----------
            qk = {b: [sb.tile([P, N], F16, tag=f"qk{b % 2}_{j}", bufs=1,
                              name=f"qk{j}") for j in range(12)]
                  for b in range(BL)}
            v = {b: [sb.tile([P, 12 * 65], F16, tag=f"v{b % 2}_{t}", bufs=1,
                             name=f"v{t}") for t in range(8)]
                 for b in range(BL)}
            at = {b: [sb.tile([P, N], F16, tag=f"at{b}_{j}", bufs=1,
                              name=f"at{b}_{j}") for j in range(6)]
                  for b in range(BL)}

            def emit_qkv_group(b, j, nh):
                qps = ps.tile([P, 512], F32, tag="mm", bufs=2, name="qps")
                for d in range(6):
                    nc.tensor.matmul(qps[:], wq_h[d][:, P * j:P * (j + 1)],
                                     xT[b][d][:, 512 * nh:512 * (nh + 1)],
                                     start=(d == 0), stop=(d == 5))
                nc.vector.tensor_scalar_add(
                    qk[b][j][:, 512 * nh:512 * (nh + 1)], qps[:],
                    bqkvT[:, j:j + 1])

            def emit_v_group(b, t, ci):
                c0, cw = ((0, 512), (512, 256))[ci]
                v3 = v[b][t].rearrange("p (h c) -> p h c", c=65)
                if ci == 0:
                    nc.vector.tensor_copy(v3[:, :, 64:65],
                                          ones_h[:, 0:12].unsqueeze(2))
                vps = ps.tile([P, 512], F32, tag="mm", bufs=2, name="vps")
                for d in range(6):
                    nc.tensor.matmul(vps[:, 0:cw],
                                     xT[b][d][:, P * t:P * (t + 1)],
                                     wq_h[d][:, 2 * D + c0:2 * D + c0 + cw],
                                     start=(d == 0), stop=(d == 5))
                nc.vector.tensor_copy(
                    v3[:, (c0 // HD):((c0 + cw) // HD), 0:HD],
                    vps[:, 0:cw].rearrange("p (h c) -> p h c", c=HD))

            # wp_h / bfinal state -- filled by fillers inside b0's window
            wp_h = []
            wpstgs = []
            bfb = sb.tile([P, D], F16, tag="bfb", bufs=1, name="bfb")

            def emit_wproj_dmas():
                for d in range(6):
                    stg = sb.tile([P, D], F32, tag="wstage", bufs=2,
                                  name="wpstg")
                    nc.scalar.dma_start(stg[:],
                                        wproj_d[P * d:P * (d + 1), :])
                    wpstgs.append(stg)

            def emit_wproj_cast(d):
                t = sb.tile([P, D], F16, tag=f"wproj{d}", bufs=1,
                            name=f"wproj{d}")
                nc.vector.tensor_copy(t[:], wpstgs[d][:])
                wp_h.append(t)

            def emit_bfinal():
                # b_final = W_proj^T b_v + b_proj  [1, 768] -> broadcast
                # to [128, 768] f16 via a DRAM bounce so the proj-output
                # copy can add it on the DVE (replaces 32 K=1 PE matmuls).
                bfinal_f = sb.tile([1, D], F32, tag="bfinal", bufs=1,
                                   name="bfinal")
                for c0, cw in ((0, 512), (512, 256)):
                    bf_ps = ps.tile([1, 512], F32, tag="mm", bufs=2,
                                    name="bf_ps")
                    for d in range(6):
                        nc.tensor.matmul(bf_ps[:, 0:cw], bv_h[:, d:d + 1],
                                         wp_h[d][:, c0:c0 + cw],
                                         start=(d == 0), stop=(d == 5))
                    nc.vector.tensor_add(bfinal_f[:, c0:c0 + cw],
                                         bf_ps[0:1, 0:cw],
                                         bproj_row[:, c0:c0 + cw])
                bfinal_h = sb.tile([1, D], F16, tag="bfinal_h", bufs=1,
                                   name="bfinal_h")
                nc.vector.tensor_copy(bfinal_h[:], bfinal_f[:])
                bf_d = dp.tile([1, D], F16, tag="bf_d", bufs=1, name="bf_d")
                nc.sync.dma_start(bf_d[:], bfinal_h[:])
                nc.sync.dma_start(bfb[:], bf_d[:].to_broadcast((P, D)))

            def emit_proj_half(b, t, ci):
                c0, cw = ((0, 512), (512, 256))[ci]
                pps = ps.tile([P, 512], F32, tag="mm", bufs=2, name="pps")
                for d in range(6):
                    nc.tensor.matmul(pps[:, 0:cw],
                                     at[b][d][:, P * t:P * (t + 1)],
                                     wp_h[d][:, c0:c0 + cw],
                                     start=(d == 0), stop=(d == 5))
                osb = sb.tile([P, 512], F32, tag="outs", bufs=2, name="osb")
                nc.vector.tensor_add(osb[:, 0:cw], pps[:, 0:cw],
                                     bfb[:, c0:c0 + cw])
                nc.sync.dma_start(out_il[b, t][:, c0:c0 + cw], osb[:, 0:cw])

            def pop_fillers(fillers, k=2):
                n = 0
                while fillers and n < k:
                    fillers.pop(0)()
                    n += 1

            # one-unit-deep queue of deferred normalize multiplies: the
            # reciprocal-broadcast DMA round trip gets a full unit of
            # latency slack instead of blocking the DVE stream in-line.
            pend_muls = []

            def flush_muls():
                while pend_muls:
                    pend_muls.pop(0)()

            def emit_unit(b, jp, nh, fillers, popk=2, tail=False):
                qt, kt = qk[b][jp], qk[b][6 + jp]
                n0 = 512 * nh
                ot = [ps.tile([65, 512], F32, tag="ot", bufs=2,
                              name="otps") for _ in range(2)]
                def emit_avs(pend):
                    for pm, pe_ in pend:
                        for hh in range(2):
                            h = 2 * jp + hh
                            nc.tensor.matmul(
                                ot[hh][:],
                                v[b][pm][:, 65 * h:65 * h + 65],
                                pe_[:, 512 * hh:512 * (hh + 1)],
                                start=(pm == 0), stop=(pm == 7))

                # m-blocks processed in pairs: the two score matmul pairs
                # are emitted back-to-back so the second pair's row-tiled
                # LDWEIGHTS pulls ahead behind a score MM (disjoint row
                # groups) instead of serializing behind a full-row AV MM.
                # AVs for the previous pair go before this pair's exps so
                # the e-buffer (bufs=3) WAR stays in emission order.
                pend = []
                for mp in range(4):
                    spss = []
                    for mi in range(2):
                        m = 2 * mp + mi
                        sps = ps.tile([P, N], F32, tag="s", bufs=2,
                                      name="sps")
                        for hh in range(2):
                            r0, r1 = HD * hh, HD * (hh + 1)
                            nc.tensor.matmul(
                                sps[:, 512 * hh:512 * (hh + 1)],
                                kt[r0:r1, P * m:P * (m + 1)],
                                qt[r0:r1, n0:n0 + 512],
                                start=True, stop=True)
                        spss.append((m, sps))
                    emit_avs(pend)
                    pend = []
                    for m, sps in spss:
                        e = sb.tile([P, N], F16, tag="e", bufs=3, name="e")
                        nc.scalar.activation(e[:], sps[:], AF.Exp,
                                             scale=SCALE)
                        pend.append((m, e))
                    if mp in (1, 2):
                        pop_fillers(fillers, popk)
                emit_avs(pend)
                # end-of-unit fillers come BEFORE the serial normalize
                # chain so their PSUM-group-closing DVE ops aren't queued
                # behind it (the next unit's filler matmuls WAR on them).
                pop_fillers(fillers, popk)
                # flush the PREVIOUS unit's deferred muls first: their bc
                # broadcasts landed a unit ago, and the bc/u_sb buffers
                # (bufs=2) are WAR-reused by this unit right after.
                flush_muls()

                def recip_chain(hh):
                    dr_f = sb.tile([1, 512], F32, tag="dr_f", bufs=1,
                                   name="dr_f")
                    nc.vector.tensor_copy(dr_f[:], ot[hh][64:65, :])
                    rr_f = sb.tile([1, 512], F32, tag="rr_f", bufs=1,
                                   name="rr_f")
                    nc.vector.reciprocal_approx_fast(out=rr_f[:],
                                                     in_=dr_f[:])
                    rr_h = sb.tile([1, 512], F16, tag="rr_h", bufs=1,
                                   name="rr_h")
                    nc.vector.tensor_copy(rr_h[:], rr_f[:])
                    rr_d = dp.tile([1, 512], F16, tag="rr_d", bufs=4,
                                   name="rr_d")
                    nc.sync.dma_start(rr_d[:], rr_h[:])
                    bc_h = sb.tile([HD, 512], F16, tag="bc_h", bufs=2,
                                   name="bc_h")
                    nc.sync.dma_start(bc_h[:],
                                      rr_d[:].to_broadcast((HD, 512)))
                    return bc_h

                def u_copy(hh):
                    # copy O^T out of PSUM promptly -- ot has no cross-
                    # unit slack (2 allocs/unit, bufs=2), so a deferred
                    # PSUM read would stall the next unit's first AV.
                    u_sb = sb.tile([HD, 512], F16, tag="u_sb", bufs=2,
                                   name="u_sb")
                    nc.vector.tensor_copy(u_sb[:], ot[hh][0:HD, :])
                    return u_sb

                if tail:
                    # window-final unit: launch both broadcast round trips
                    # before the O^T copies -- the tail projection waits
                    # on the muls, so the DMA latency is critical here.
                    bcs = [recip_chain(0), recip_chain(1)]
                    us = [u_copy(0), u_copy(1)]
                else:
                    us, bcs = [], []
                    for hh in range(2):
                        us.append(u_copy(hh))
                        bcs.append(recip_chain(hh))
                for hh in range(2):
                    r0, r1 = HD * hh, HD * (hh + 1)
                    pend_muls.append(
                        lambda u=us[hh], bc=bcs[hh], r0=r0, r1=r1:
                        nc.vector.tensor_mul(at[b][jp][r0:r1, n0:n0 + 512],
                                             u[:], bc[:]))

            # ---------- startup: b0 transposes + QKV pair 0 + V(0,1);
            # V(2..7) ride as unit-0 fillers (popk=4 keeps each V(t)
            # ahead of the AV matmul that consumes it)
            for t in range(8):
                emit_transpose_chunk(0, t)
            for t in range(8):
                emit_v_group(0, t, 0)
                emit_v_group(0, t, 1)
            emit_qkv_group(0, 0, 0)
            emit_qkv_group(0, 6, 0)
            emit_qkv_group(0, 6, 1)

            # cast the late W_qkv columns on DVE (reads must be emitted
            # before the wproj DMAs reuse the wstage buffers)
            for d in range(6):
                for rg in range(2):
                    c0 = (P, 7 * P)[rg]
                    nc.vector.tensor_copy(wq_h[d][:, c0:c0 + 5 * P],
                                          wr_stgs[2 * d + rg][:])

            # issue remaining bulk DMAs now (land well before their use)
            emit_wproj_dmas()
            for c in range(3):
                emit_x_dma(1, c)

            # ---------- b0 attention, nh-outer ----------
            # nh0 fillers: pair jp+1's q/k groups front-loaded so each
            # pair's inputs are emitted a unit ahead of use (per-engine
            # streams are in-order: a consumer emitted before its producer
            # would deadlock), then q(*,nh1) groups, wproj casts, bfinal.
            q_nh0 = []
            for jpn in range(1, 6):
                q_nh0 += [
                    lambda j=jpn: emit_qkv_group(0, j, 0),
                    lambda j=6 + jpn: emit_qkv_group(0, j, 0),
                    lambda j=6 + jpn: emit_qkv_group(0, j, 1),
                    lambda j=jpn - 1: emit_qkv_group(0, j, 1),
                ]
            q_nh0 += [lambda: emit_qkv_group(0, 5, 1)]
            q_nh0 += [lambda d=d: emit_wproj_cast(d) for d in range(6)]
            q_nh0 += [emit_bfinal]
            for jp in range(6):
                emit_unit(0, jp, 0, q_nh0, tail=(jp == 5))
            flush_muls()
            while q_nh0:
                q_nh0.pop(0)()

            # nh1 fillers: b1 transposes + V(1) interleaved per chunk,
            # b1 QKV pair 0, then b0 proj for nh0 token chunks (0-3).
            q_nh1 = []
            for t in range(8):
                q_nh1 += [lambda t=t: emit_transpose_chunk(1, t)]
                q_nh1 += [lambda t=t, ci=ci: emit_v_group(1, t, ci)
                          for ci in range(2)]
            q_nh1 += [lambda: emit_qkv_group(1, 0, 0),
                      lambda: emit_qkv_group(1, 6, 0),
                      lambda: emit_qkv_group(1, 6, 1)]
            q_nh1 += [lambda t=t, ci=ci: emit_proj_half(0, t, ci)
                      for t in range(4) for ci in range(2)]
            for jp in range(6):
                emit_unit(0, jp, 1, q_nh1, tail=(jp == 5))
            flush_muls()
            while q_nh1:
                q_nh1.pop(0)()

            # ---------- b1 attention, nh-outer ----------
            # nh0 fillers: b1 QKV pairs 1-5 staged one unit ahead, then b0
            # proj for nh1 token chunks (popped at units 2+, after which
            # the at[0] reads are done so b1's at writes don't stall).
            q1_nh0 = [lambda: emit_qkv_group(1, 0, 1)]
            for jpn in range(1, 6):
                q1_nh0 += [
                    lambda j=jpn: emit_qkv_group(1, j, 0),
                    lambda j=6 + jpn: emit_qkv_group(1, j, 0),
                    lambda j=6 + jpn: emit_qkv_group(1, j, 1),
                    lambda j=jpn: emit_qkv_group(1, j, 1),
                ]
            for jp in range(6):
                emit_unit(1, jp, 0, q1_nh0, tail=(jp == 5))
            flush_muls()
            while q1_nh0:
                q1_nh0.pop(0)()

            # nh1 fillers: remaining b0 proj (at tags are per-batch now,
            # so these reads no longer alias b1's at writes), then b1 proj
            # for nh0 token chunks (0-3).
            q1_nh1 = [lambda t=t, ci=ci: emit_proj_half(0, t, ci)
                      for t in range(4, 8) for ci in range(2)]
            q1_nh1 += [lambda t=t, ci=ci: emit_proj_half(1, t, ci)
                      for t in range(4) for ci in range(2)]
            for jp in range(6):
                emit_unit(1, jp, 1, q1_nh1, tail=(jp == 5))
            flush_muls()
            while q1_nh1:
                q1_nh1.pop(0)()

            # ---------- tail: b1 proj for nh1 token chunks ----------
            for t in range(4, 8):
                for ci in range(2):
                    emit_proj_half(1, t, ci)
    nc.compile()
    return nc


def _get_nc():
    if "nc" not in _CACHE:
        _CACHE["nc"] = _build()
    return _CACHE["nc"]


def kernel(x, W_qkv, b_qkv, W_proj, b_proj):
    from concourse.bass_utils import run_bass_kernel_spmd

    nc = _get_nc()
    x = np.ascontiguousarray(x, dtype=np.float32)
    in_maps = [
        {
            "x": x[2 * i:2 * i + 2],
            "W_qkv": np.asarray(W_qkv, dtype=np.float32),
            "b_qkv": np.asarray(b_qkv, dtype=np.float32),
            "W_proj": np.asarray(W_proj, dtype=np.float32),
            "b_proj": np.asarray(b_proj, dtype=np.float32),
        }
        for i in range(8)
    ]
    res = run_bass_kernel_spmd(nc, in_maps, core_ids=list(range(8)))
    return np.concatenate([r["out"] for r in res.results], axis=0)


# revision 50
# speedup vs baseline: 1.1030x; 1.0084x over previous
"""Multi-head self-attention (B=16, N=1024, D=768, H=12) on 8 TRN2 NeuronCores.

Data-parallel over batch (2 batches per core, weights replicated, no
collectives). Per core, one fused Bass/Tile kernel:

  x --8 chunked interleaved DMAs--> x6 [128, 8*768] (token 8p+t on
      partition p, slot t; attention is permutation-invariant over tokens,
      so the interleave is only undone at the output DMA)
  x6 --f16 cast + PE transpose--> xT [d, tok]
  QT/KT = (W_qkv^T x^T + b) in [col, tok] layout (f16)
  V_aug = [x W_v | ones-col per head]  [tok, 12*65] (f16)
  per head: S^T[m,n] = K Q^T (PE), E = exp(S^T*scale) (ACT, [128,1024]),
      O^T = V_aug^T E (PE; row 64 = softmax denominator via the ones
      column -- no max subtraction needed, scores are O(1)).
      normalize: recip_approx_fast straight from PSUM row 64,
      DMA-broadcast the reciprocal row (f32), one DVE mul reading PSUM.
  out = attnT^T W_proj, bias (W_proj^T b_v + b_proj) added by the DVE
      during the PSUM->SBUF copy from a DMA-broadcast bias tile
      (V-bias folded through softmax since rows of A sum to 1)

All matmul operands f16 (1 cycle/row; fp32/f32r run 2-pass fp32_mode=HIGH
at 1/4 rate and break HAM warm-up -- measured). PSUM accumulation is f32.

Scheduling: engines execute fixed in-order streams, so emission order IS
the schedule. Startup pipelines chunked x DMAs -> transposes -> V/QKV so
batch-0 attention starts ~30us earlier than a phase-serial ordering.
Attention loops are nh-outer so each batch's first-half projection becomes
filler work inside its own attention window; b1's QKV pairs 1-5 are
deferred into b1's window to keep the PE fed there (stalled PE breaks the
LDWEIGHTS pull-ahead and inflates score MMs from 213ns to 318ns).
"""

import numpy as np

_CACHE: dict = {}

P = 128
BL, N, D, H, HD = 2, 1024, 768, 12, 64
D3 = 3 * D
SCALE = float(HD) ** -0.5


def _build():
    import concourse.mybir as mybir
    import concourse.tile as tile
    from concourse import bacc
    from concourse.masks import make_identity

    dt = mybir.dt
    F32, F16 = dt.float32, dt.float16
    AF = mybir.ActivationFunctionType

    nc = bacc.Bacc("TRN2", target_bir_lowering=False, debug=False)
    x_d = nc.dram_tensor("x", [BL, N, D], F32, kind="ExternalInput").ap()
    wqkv_d = nc.dram_tensor("W_qkv", [D, D3], F32, kind="ExternalInput").ap()
    bqkv_d = nc.dram_tensor("b_qkv", [D3], F32, kind="ExternalInput").ap()
    wproj_d = nc.dram_tensor("W_proj", [D, D], F32, kind="ExternalInput").ap()
    bproj_d = nc.dram_tensor("b_proj", [D], F32, kind="ExternalInput").ap()
    out_d = nc.dram_tensor("out", [BL, N, D], F32, kind="ExternalOutput").ap()
    # token-interleaved views: partition p, slot t <-> token 8p+t
    x_il = x_d.rearrange("b (p i) d -> b p (i d)", p=P)       # [2, 128, 6144]
    out_il = out_d.rearrange("b (p i) d -> b i p d", p=P)     # [2, 8, 128, 768]

    with tile.TileContext(nc) as tc:
        with tc.tile_pool(name="sb", bufs=1) as sb, \
             tc.tile_pool(name="dp", bufs=1, space="DRAM") as dp, \
             tc.tile_pool(name="ps", bufs=2, space="PSUM") as ps:

            # ---------- constants ----------
            ident = sb.tile([P, P], F16, tag="ident", bufs=1, name="ident")
            make_identity(nc, ident[:])
            ones_h = sb.tile([P, P], F16, tag="ones_h", bufs=1, name="ones_h")
            nc.vector.memset(ones_h[:], 1.0)

            # ---------- DMA issue order, by first use: V columns of W_qkv,
            # x chunks 0-3, the j=0/j=6 W columns (QKV pair 0), x chunks
            # 4-7, then the remaining W columns (pairs 1-5, used as b0-
            # window fillers so they may land late).
            # x staged per 768-col slot through a rotating 3-buffer pool
            # (saves 15KB/partition vs a monolithic [128, 6144] tile; the
            # freed space un-aliases the two batches' at tiles below).
            xstg = {}

            def emit_x_dma(b, t):
                stg = sb.tile([P, D], F32, tag="xstage", bufs=3,
                              name="xstg")
                nc.sync.dma_start(stg[:], x_il[b][:, D * t:D * (t + 1)])
                xstg[(b, t)] = stg

            for c in range(2):
                emit_x_dma(0, c)

            wv_stgs = []
            for d in range(6):
                stg = sb.tile([P, D], F32, tag="wstage", bufs=2,
                              name="wvstg")
                nc.sync.dma_start(stg[:],
                                  wqkv_d[P * d:P * (d + 1), 2 * D:3 * D])
                wv_stgs.append(stg)

            emit_x_dma(0, 2)

            wj_stgs = []
            for d in range(6):
                for j in (0, 6):
                    stg = sb.tile([P, P], F32, tag="wjstage", bufs=3,
                                  name="wjstg")
                    nc.sync.dma_start(
                        stg[:], wqkv_d[P * d:P * (d + 1),
                                       P * j:P * (j + 1)])
                    wj_stgs.append(stg)


            # bias + bproj staging on the scalar DMA queue
            bstg = sb.tile([18, P], F32, tag="bstg", bufs=1, name="bstg")
            nc.scalar.dma_start(bstg[:], bqkv_d.rearrange("(j p) -> j p", p=P))
            bproj_row = sb.tile([1, D], F32, tag="bproj_row", bufs=1,
                                name="bproj_row")
            nc.scalar.dma_start(bproj_row[:], bproj_d.unsqueeze(0))

            # ---------- PE warm-up: ~4us of dummy matmuls flips HAM to 8/8
            # (transposes run in transpose-mode, which does not warm HAM)
            warm_h = sb.tile([P, 512], F16, tag="e", bufs=4, name="warm_h")
            nc.vector.memset(warm_h[:], 0.0)
            for wi in range(10):
                wps = ps.tile([P, 512], F32, tag="mm", bufs=2, name="wps")
                nc.tensor.matmul(wps[:], ones_h[:, 0:P], warm_h[:],
                                 start=True, stop=True)

            # ---------- W_qkv casts: V cols + j0/j6 on the (startup-idle)
            # scalar engine; the rest on DVE later (emission-order note:
            # these reads must precede the wrest DMAs' tag reuse).
            wq_h = []
            for d in range(6):
                t = sb.tile([P, D3], F16, tag=f"wqkv{d}", bufs=1,
                            name=f"wqkv{d}")
                nc.scalar.copy(t[:, 2 * D:3 * D], wv_stgs[d][:])
                for ji, j in enumerate((0, 6)):
                    nc.scalar.copy(t[:, P * j:P * (j + 1)],
                                   wj_stgs[2 * d + ji][:])
                wq_h.append(t)

            # remaining W_qkv columns (lands late; cast on DVE at startup
            # end, consumed by pair-1..5 QKV groups inside b0's window)
            wr_stgs = []
            for d in range(6):
                for rg in range(2):
                    c0 = (P, 7 * P)[rg]
                    stg = sb.tile([P, 5 * P], F32, tag="wstage", bufs=2,
                                  name="wrstg")
                    nc.sync.dma_start(
                        stg[:], wqkv_d[P * d:P * (d + 1), c0:c0 + 5 * P])
                    wr_stgs.append(stg)

            # ---------- bias pipeline (DVE + one PE transpose) ----------
            bstg_h = sb.tile([18, P], F16, tag="bstg_h", bufs=1, name="bstg_h")
            nc.vector.tensor_copy(bstg_h[:], bstg[:])
            btp = ps.tile([P, 18], F16, tag="mm", bufs=2, name="btp")
            nc.tensor.transpose(btp[:], bstg_h[:], ident[0:18, 0:18])
            bqkvT = sb.tile([P, 18], F32, tag="bqkvT", bufs=1, name="bqkvT")
            nc.vector.tensor_copy(bqkvT[:], btp[:])
            bv_h = sb.tile([P, 6], F16, tag="bv_h", bufs=1, name="bv_h")
            nc.vector.tensor_copy(bv_h[:], btp[:, 12:18])

            xT = {b: [sb.tile([P, N], F16, tag=f"xT{b}_{j}", bufs=1,
                              name=f"xT{b}_{j}") for j in range(6)]
                  for b in range(BL)}

            def emit_transpose_chunk(b, t):
                xh = sb.tile([P, D], F16, tag="xh", bufs=1, name="xh")
                nc.vector.tensor_copy(xh[:], xstg[(b, t)][:])
                if t + 3 < 8:
                    emit_x_dma(b, t + 3)
                for j in range(6):
                    tp = ps.tile([P, P], F16, tag="mm", bufs=2, name="tp")
                    nc.tensor.transpose(tp[:], xh[:, P * j:P * (j + 1)],
                                        ident[:])
                    nc.vector.tensor_copy(xT[b][j][:, P * t:P * (t + 1)],
                                          tp[:])

            # ---------- result tiles ----------
            qk = {b: [sb.tile([P, N], F16, tag=f"qk{b % 2}_{j}", bufs=1,
                              name=f"qk{j}") for j in range(12)]
                  for b in range(BL)}
            v = {b: [sb.tile([P, 12 * 65], F16, tag=f"v{b % 2}_{t}", bufs=1,
                             name=f"v{t}") for t in range(8)]
                 for b in range(BL)}
            at = {b: [sb.tile([P, N], F16, tag=f"at{b}_{j}", bufs=1,
                              name=f"at{b}_{j}") for j in range(6)]
                  for b in range(BL)}

            def emit_qkv_group(b, j, nh):
                qps = ps.tile([P, 512], F32, tag="mm", bufs=2, name="qps")
                for d in range(6):
                    nc.tensor.matmul(qps[:], wq_h[d][:, P * j:P * (j + 1)],
                                     xT[b][d][:, 512 * nh:512 * (nh + 1)],
                                     start=(d == 0), stop=(d == 5))
                nc.vector.tensor_scalar_add(
                    qk[b][j][:, 512 * nh:512 * (nh + 1)], qps[:],
                    bqkvT[:, j:j + 1])

            def emit_v_group(b, t, ci):
                c0, cw = ((0, 512), (512, 256))[ci]
                v3 = v[b][t].rearrange("p (h c) -> p h c", c=65)
                if ci == 0:
                    nc.vector.tensor_copy(v3[:, :, 64:65],
                                          ones_h[:, 0:12].unsqueeze(2))
                vps = ps.tile([P, 512], F32, tag="mm", bufs=2, name="vps")
                for d in range(6):
                    nc.tensor.matmul(vps[:, 0:cw],
                                     xT[b][d][:, P * t:P * (t + 1)],
                                     wq_h[d][:, 2 * D + c0:2 * D + c0 + cw],
                                     start=(d == 0), stop=(d == 5))
                nc.vector.tensor_copy(
                    v3[:, (c0 // HD):((c0 + cw) // HD), 0:HD],
                    vps[:, 0:cw].rearrange("p (h c) -> p h c", c=HD))

            # wp_h / bfinal state -- filled by fillers inside b0's window
            wp_h = []
            wpstgs = []
            bfb = sb.tile([P, D], F16, tag="bfb", bufs=1, name="bfb")

            def emit_wproj_dmas():
                for d in range(6):
                    stg = sb.tile([P, D], F32, tag="wstage", bufs=2,
                                  name="wpstg")
                    nc.scalar.dma_start(stg[:],
                                        wproj_d[P * d:P * (d + 1), :])
                    wpstgs.append(stg)

            def emit_wproj_cast(d):
                t = sb.tile([P, D], F16, tag=f"wproj{d}", bufs=1,
                            name=f"wproj{d}")
                nc.vector.tensor_copy(t[:], wpstgs[d][:])
                wp_h.append(t)

            def emit_bfinal():
                # b_final = W_proj^T b_v + b_proj  [1, 768] -> broadcast
                # to [128, 768] f16 via a DRAM bounce so the proj-output
                # copy can add it on the DVE (replaces 32 K=1 PE matmuls).
                bfinal_f = sb.tile([1, D], F32, tag="bfinal", bufs=1,
                                   name="bfinal")
                for c0, cw in ((0, 512), (512, 256)):
                    bf_ps = ps.tile([1, 512], F32, tag="mm", bufs=2,
                                    name="bf_ps")
                    for d in range(6):
                        nc.tensor.matmul(bf_ps[:, 0:cw], bv_h[:, d:d + 1],
                                         wp_h[d][:, c0:c0 + cw],
                                         start=(d == 0), stop=(d == 5))
                    nc.vector.tensor_add(bfinal_f[:, c0:c0 + cw],
                                         bf_ps[0:1, 0:cw],
                                         bproj_row[:, c0:c0 + cw])
                bfinal_h = sb.tile([1, D], F16, tag="bfinal_h", bufs=1,
                                   name="bfinal_h")
                nc.vector.tensor_copy(bfinal_h[:], bfinal_f[:])
                bf_d = dp.tile([1, D], F16, tag="bf_d", bufs=1, name="bf_d")
                nc.sync.dma_start(bf_d[:], bfinal_h[:])
                nc.sync.dma_start(bfb[:], bf_d[:].to_broadcast((P, D)))

            def emit_proj_half(b, t, ci):
                c0, cw = ((0, 512), (512, 256))[ci]
                pps = ps.tile([P, 512], F32, tag="mm", bufs=2, name="pps")
                for d in range(6):
                    nc.tensor.matmul(pps[:, 0:cw],
                                     at[b][d][:, P * t:P * (t + 1)],
                                     wp_h[d][:, c0:c0 + cw],
                                     start=(d == 0), stop=(d == 5))
                osb = sb.tile([P, 512], F32, tag="outs", bufs=2, name="osb")
                nc.vector.tensor_add(osb[:, 0:cw], pps[:, 0:cw],
                                     bfb[:, c0:c0 + cw])
                nc.sync.dma_start(out_il[b, t][:, c0:c0 + cw], osb[:, 0:cw])

            def pop_fillers(fillers, k=2):
                n = 0
                while fillers and n < k:
                    fillers.pop(0)()
                    n += 1

            # one-unit-deep queue of deferred normalize multiplies: the
            # reciprocal-broadcast DMA round trip gets a full unit of
            # latency slack instead of blocking the DVE stream in-line.
            pend_muls = []

            def flush_muls():
                while pend_muls:
                    pend_muls.pop(0)()

            def emit_unit(b, jp, nh, fillers, popk=2, tail=False):
                qt, kt = qk[b][jp], qk[b][6 + jp]
                n0 = 512 * nh
                ot = [ps.tile([65, 512], F32, tag="ot", bufs=2,
                              name="otps") for _ in range(2)]
                def emit_avs(pend):
                    for pm, pe_ in pend:
                        for hh in range(2):
                            h = 2 * jp + hh
                            nc.tensor.matmul(
                                ot[hh][:],
                                v[b][pm][:, 65 * h:65 * h + 65],
                                pe_[:, 512 * hh:512 * (hh + 1)],
                                start=(pm == 0), stop=(pm == 7))

                # m-blocks processed in pairs: the two score matmul pairs
                # are emitted back-to-back so the second pair's row-tiled
                # LDWEIGHTS pulls ahead behind a score MM (disjoint row
                # groups) instead of serializing behind a full-row AV MM.
                # AVs for the previous pair go before this pair's exps so
                # the e-buffer (bufs=3) WAR stays in emission order.
                pend = []
                for mp in range(4):
                    spss = []
                    for mi in range(2):
                        m = 2 * mp + mi
                        sps = ps.tile([P, N], F32, tag="s", bufs=2,
                                      name="sps")
                        for hh in range(2):
                            r0, r1 = HD * hh, HD * (hh + 1)
                            nc.tensor.matmul(
                                sps[:, 512 * hh:512 * (hh + 1)],
                                kt[r0:r1, P * m:P * (m + 1)],
                                qt[r0:r1, n0:n0 + 512],
                                start=True, stop=True)
                        spss.append((m, sps))
                    emit_avs(pend)
                    pend = []
                    for m, sps in spss:
                        e = sb.tile([P, N], F16, tag="e", bufs=4, name="e")
                        nc.scalar.activation(e[:], sps[:], AF.Exp,
                                             scale=SCALE)
                        pend.append((m, e))
                    if mp in (1, 2):
                        pop_fillers(fillers, popk)
                emit_avs(pend)
                # end-of-unit fillers come BEFORE the serial normalize
                # chain so their PSUM-group-closing DVE ops aren't queued
                # behind it (the next unit's filler matmuls WAR on them).
                pop_fillers(fillers, popk)
                # flush the PREVIOUS unit's deferred muls first: their bc
                # broadcasts landed a unit ago, and the bc/u_sb buffers
                # (bufs=2) are WAR-reused by this unit right after.
                flush_muls()

                def recip_chain(hh):
                    dr_f = sb.tile([1, 512], F32, tag="dr_f", bufs=1,
                                   name="dr_f")
                    nc.vector.tensor_copy(dr_f[:], ot[hh][64:65, :])
                    rr_f = sb.tile([1, 512], F32, tag="rr_f", bufs=1,
                                   name="rr_f")
                    nc.vector.reciprocal_approx_fast(out=rr_f[:],
                                                     in_=dr_f[:])
                    rr_h = sb.tile([1, 512], F16, tag="rr_h", bufs=1,
                                   name="rr_h")
                    nc.vector.tensor_copy(rr_h[:], rr_f[:])
                    rr_d = dp.tile([1, 512], F16, tag="rr_d", bufs=4,
                                   name="rr_d")
                    nc.sync.dma_start(rr_d[:], rr_h[:])
                    bc_h = sb.tile([HD, 512], F16, tag="bc_h", bufs=2,
                                   name="bc_h")
                    nc.sync.dma_start(bc_h[:],
                                      rr_d[:].to_broadcast((HD, 512)))
                    return bc_h

                def u_copy(hh):
                    # copy O^T out of PSUM promptly -- ot has no cross-
                    # unit slack (2 allocs/unit, bufs=2), so a deferred
                    # PSUM read would stall the next unit's first AV.
                    u_sb = sb.tile([HD, 512], F16, tag="u_sb", bufs=2,
                                   name="u_sb")
                    nc.vector.tensor_copy(u_sb[:], ot[hh][0:HD, :])
                    return u_sb

                if tail:
                    # window-final unit: launch both broadcast round trips
                    # before the O^T copies -- the tail projection waits
                    # on the muls, so the DMA latency is critical here.
                    bcs = [recip_chain(0), recip_chain(1)]
                    us = [u_copy(0), u_copy(1)]
                else:
                    us, bcs = [], []
                    for hh in range(2):
                        us.append(u_copy(hh))
                        bcs.append(recip_chain(hh))
                for hh in range(2):
                    r0, r1 = HD * hh, HD * (hh + 1)
                    pend_muls.append(
                        lambda u=us[hh], bc=bcs[hh], r0=r0, r1=r1:
                        nc.vector.tensor_mul(at[b][jp][r0:r1, n0:n0 + 512],
                                             u[:], bc[:]))

            # ---------- startup: b0 transposes + QKV pair 0 + V(0,1);
            # V(2..7) ride as unit-0 fillers (popk=4 keeps each V(t)
            # ahead of the AV matmul that consumes it)
            for t in range(8):
                emit_transpose_chunk(0, t)
            for t in range(8):
                emit_v_group(0, t, 0)
                emit_v_group(0, t, 1)
            emit_qkv_group(0, 0, 0)
            emit_qkv_group(0, 6, 0)
            emit_qkv_group(0, 6, 1)

            # cast the late W_qkv columns on DVE (reads must be emitted
            # before the wproj DMAs reuse the wstage buffers)
            for d in range(6):
                for rg in range(2):
                    c0 = (P, 7 * P)[rg]
                    nc.vector.tensor_copy(wq_h[d][:, c0:c0 + 5 * P],
                                          wr_stgs[2 * d + rg][:])

            # issue remaining bulk DMAs now (land well before their use)
            emit_wproj_dmas()
            for c in range(3):
                emit_x_dma(1, c)

            # ---------- b0 attention, nh-outer ----------
            # nh0 fillers: pair jp+1's q/k groups front-loaded so each
            # pair's inputs are emitted a unit ahead of use (per-engine
            # streams are in-order: a consumer emitted before its producer
            # would deadlock), then q(*,nh1) groups, wproj casts, bfinal.
            q_nh0 = []
            for jpn in range(1, 6):
                q_nh0 += [
                    lambda j=jpn: emit_qkv_group(0, j, 0),
                    lambda j=6 + jpn: emit_qkv_group(0, j, 0),
                    lambda j=6 + jpn: emit_qkv_group(0, j, 1),
                    lambda j=jpn - 1: emit_qkv_group(0, j, 1),
                ]
            q_nh0 += [lambda: emit_qkv_group(0, 5, 1)]
            q_nh0 += [lambda d=d: emit_wproj_cast(d) for d in range(6)]
            q_nh0 += [emit_bfinal]
            for jp in range(6):
                emit_unit(0, jp, 0, q_nh0, tail=(jp == 5))
            flush_muls()
            while q_nh0:
                q_nh0.pop(0)()

            # nh1 fillers: b1 transposes + V(1) interleaved per chunk,
            # b1 QKV pair 0, then b0 proj for nh0 token chunks (0-3).
            q_nh1 = []
            for t in range(8):
                q_nh1 += [lambda t=t: emit_transpose_chunk(1, t)]
                q_nh1 += [lambda t=t, ci=ci: emit_v_group(1, t, ci)
                          for ci in range(2)]
            q_nh1 += [lambda: emit_qkv_group(1, 0, 0),
                      lambda: emit_qkv_group(1, 6, 0),
                      lambda: emit_qkv_group(1, 6, 1)]
            q_nh1 += [lambda t=t, ci=ci: emit_proj_half(0, t, ci)
                      for t in range(4) for ci in range(2)]
            for jp in range(6):
                emit_unit(0, jp, 1, q_nh1, tail=(jp == 5))
            flush_muls()
            while q_nh1:
                q_nh1.pop(0)()

            # ---------- b1 attention, nh-outer ----------
            # nh0 fillers: b1 QKV pairs 1-5 staged one unit ahead, then b0
            # proj for nh1 token chunks (popped at units 2+, after which
            # the at[0] reads are done so b1's at writes don't stall).
            q1_nh0 = [lambda: emit_qkv_group(1, 0, 1)]
            for jpn in range(1, 6):
                q1_nh0 += [
                    lambda j=jpn: emit_qkv_group(1, j, 0),
                    lambda j=6 + jpn: emit_qkv_group(1, j, 0),
                    lambda j=6 + jpn: emit_qkv_group(1, j, 1),
                    lambda j=jpn: emit_qkv_group(1, j, 1),
                ]
            for jp in range(6):
                emit_unit(1, jp, 0, q1_nh0, tail=(jp == 5))
            flush_muls()
            while q1_nh0:
                q1_nh0.pop(0)()

            # nh1 fillers: remaining b0 proj (at tags are per-batch now,
            # so these reads no longer alias b1's at writes), then b1 proj
            # for nh0 token chunks (0-3).
            q1_nh1 = [lambda t=t, ci=ci: emit_proj_half(0, t, ci)
                      for t in range(4, 8) for ci in range(2)]
            q1_nh1 += [lambda t=t, ci=ci: emit_proj_half(1, t, ci)
                      for t in range(4) for ci in range(2)]
            for jp in range(6):
                emit_unit(1, jp, 1, q1_nh1, popk=1, tail=(jp == 5))
            flush_muls()
            while q1_nh1:
                q1_nh1.pop(0)()

            # ---------- tail: b1 proj for nh1 token chunks ----------
            for t in range(4, 8):
                for ci in range(2):
                    emit_proj_half(1, t, ci)
    nc.compile()
    return nc


def _get_nc():
    if "nc" not in _CACHE:
        _CACHE["nc"] = _build()
    return _CACHE["nc"]


def kernel(x, W_qkv, b_qkv, W_proj, b_proj):
    from concourse.bass_utils import run_bass_kernel_spmd

    nc = _get_nc()
    x = np.ascontiguousarray(x, dtype=np.float32)
    in_maps = [
        {
            "x": x[2 * i:2 * i + 2],
            "W_qkv": np.asarray(W_qkv, dtype=np.float32),
            "b_qkv": np.asarray(b_qkv, dtype=np.float32),
            "W_proj": np.asarray(W_proj, dtype=np.float32),
            "b_proj": np.asarray(b_proj, dtype=np.float32),
        }
        for i in range(8)
    ]
    res = run_bass_kernel_spmd(nc, in_maps, core_ids=list(range(8)))
    return np.concatenate([r["out"] for r in res.results], axis=0)
